# revision 21
# baseline (speedup 1.0000x reference)
"""Distributed Bass kernel for fused GQA attention block (ANEFullAttention).

Full op: qkv-proj (wq also produces a sigmoid gate), q/k rmsnorm, partial
interleaved RoPE (first 32 of 128 dims), causal GQA attention (16 q heads /
4 kv heads), gate multiply, o_proj; returns (out, k_cache, v_cache).

Sharding over 8 cores: core c -> (batch b = c//4, head-group g = c%4).
Each core owns 4 q heads + 1 kv head of one batch.  o_proj is handled by an
8-way AllToAll (split in two halves by head pair, so the first half overlaps
the remaining attention work): each core ships its gated attention output
(transposed, [d_local, S]) sliced into 8 s-shards; afterwards core j holds
the full 2048-dim attention output for BOTH batches on s-rows
[j*S/8,(j+1)*S/8) and computes that slice of o_proj against replicated wo.

Perf notes (measured on TRN2):
- All weight/activation DRAM parameters are pre-packed on the host into the
  exact [128, ...contiguous-free-dim] device layout so every HWDGE dma_start
  is a cheap 2D pattern (multi-dim APs are descriptor-generated inline on
  the issuing sequencer at ~5-12us per call).
- ScalarE(ACT) runs only Exp + batched Sigmoid/Sqrt (activation-table swaps
  cost ~1.3us); DVE does PSUM evictions, norm scaling, rope, masks, gating.
- q heads and the kv head share one fused norm/rope pipeline (wq columns are
  host-reordered to [q0..q3 | gate0..gate3] so q|k rows are contiguous).
- walrus runs with ldw-opt disabled: every matmul pays its own LDWEIGHTS,
  so matmul count is kept minimal and moving dims maximal.
"""

import os
import sys

_TRN_REPO = "/opt/trn_rl_repo"
if _TRN_REPO not in sys.path:
    sys.path.insert(0, _TRN_REPO)

import numpy as np
import ml_dtypes

# ---------------------------------------------------------------- config

FULL_CFG = dict(
    B=2, S=2048, HID=2048, NH=16, NKV=4, HD=128, ROT=32,
    THETA=10000000.0, EPS=1e-6,
)


def _derived(cfg):
    d = dict(cfg)
    d["GC"] = 4                       # head-groups (tensor-parallel degree)
    d["NCORES"] = 8
    d["NHL"] = d["NH"] // d["GC"]     # q heads per core
    d["KO"] = d["HID"] // 128         # contraction blocks for projections
    d["SB"] = d["S"] // 128           # 128-row s blocks
    d["BAND"] = min(512, d["S"])      # moving-dim width for score matmuls
    d["NBAND"] = d["S"] // d["BAND"]
    d["TPB"] = d["BAND"] // 128       # t-blocks per band
    d["SQ"] = d["S"] // d["NCORES"]   # per-core o_proj s-slice (per batch)
    d["DM"] = d["NH"] * d["HD"]       # attention model dim (o_proj contraction)
    d["KO2"] = d["DM"] // 128
    d["NOB"] = (d["HID"] + 511) // 512
    d["SCALE"] = d["HD"] ** -0.5
    return d


# ---------------------------------------------------------------- builder

def build_nc(cfg):
    import concourse.bass as bass
    import concourse.tile as tile
    import concourse.mybir as mybir
    from concourse import bacc
    from concourse.masks import make_identity

    c = _derived(cfg)
    S, HID, HD, ROT = c["S"], c["HID"], c["HD"], c["ROT"]
    NHL, KO, SB = c["NHL"], c["KO"], c["SB"]
    BAND, NBAND, TPB, SQ = c["BAND"], c["NBAND"], c["TPB"], c["SQ"]
    KO2, NOB, SCALE, EPS = c["KO2"], c["NOB"], c["SCALE"], c["EPS"]
    NC = c["NCORES"]
    NQG = NHL * 2 * HD                # 1024 (q heads then gates)
    N5 = NHL + 1                      # q heads + the kv head
    RH = ROT // 2

    f32 = mybir.dt.float32
    bf16 = mybir.dt.bfloat16
    AF = mybir.ActivationFunctionType
    ALU = mybir.AluOpType

    nc = bacc.Bacc(None, target_bir_lowering=False, debug=False, num_devices=NC)

    # -------- dram parameters (names = in_map keys; all pre-packed 2D)
    xT = nc.declare_dram_parameter("xT", [128, SB * KO * 128], bf16, isOutput=False)
    wq = nc.declare_dram_parameter("wq", [128, KO * NQG], bf16, isOutput=False)
    wkv = nc.declare_dram_parameter("wkv", [128, KO * 2 * HD], bf16, isOutput=False)
    wo = nc.declare_dram_parameter("wo", [128, NOB * KO2 * 512], bf16, isOutput=False)
    cos5 = nc.declare_dram_parameter("cos5", [128, SB * N5 * RH], f32, isOutput=False)
    sin5 = nc.declare_dram_parameter("sin5", [128, SB * N5 * RH], f32, isOutput=False)
    w1qk = nc.declare_dram_parameter("w1qk", [128, N5 * HD], f32, isOutput=False)
    tri = nc.declare_dram_parameter("tri", [128, 128], bf16, isOutput=False)

    out = nc.declare_dram_parameter("out", [2, SQ, HID], f32, isOutput=True)
    k_cache = nc.declare_dram_parameter("k_cache", [S, HD], f32, isOutput=True)
    v_cache = nc.declare_dram_parameter("v_cache", [S, HD], f32, isOutput=True)

    with tile.TileContext(nc) as tc:
        with tc.tile_pool(name="const", bufs=1) as const, \
             tc.tile_pool(name="persist", bufs=1) as persist, \
             tc.tile_pool(name="dram", bufs=1, space="DRAM") as dram:
            ident_b = const.tile([128, 128], bf16)
            make_identity(nc, ident_b)

            w1qk_sb = const.tile([128, N5, HD], f32)
            nc.gpsimd.dma_start(
                out=w1qk_sb[:], in_=w1qk.rearrange("p (h d) -> p h d", h=N5)
            )
            tri_sb = const.tile([128, 128], bf16)
            nc.gpsimd.dma_start(out=tri_sb[:], in_=tri[:, :])
            eps_sb = const.tile([128, 1], f32)
            nc.vector.memset(eps_sb[:], EPS)

            cos5_sb = const.tile([128, SB, N5, RH], f32)
            nc.gpsimd.dma_start(
                out=cos5_sb[:],
                in_=cos5.rearrange("p (sb h f) -> p sb h f", sb=SB, h=N5),
            )
            sin5_sb = const.tile([128, SB, N5, RH], f32)
            nc.gpsimd.dma_start(
                out=sin5_sb[:],
                in_=sin5.rearrange("p (sb h f) -> p sb h f", sb=SB, h=N5),
            )

            # phase-1 -> phase-2 tensors
            qT_sb = persist.tile([128, NHL, S], bf16)      # [d, h, s]
            kT_sb = persist.tile([128, S], bf16)           # [d, t]
            v_sb = persist.tile([128, SB, HD + 1], bf16)   # [t_lo, t_blk, d+ones]
            gate_sb = persist.tile([128, SB, NHL, HD], bf16)
            nc.vector.memset(v_sb[:, :, HD : HD + 1], 1.0)

            # a2a bounce buffers, split in half by head; rows (p*HSPLIT + h)
            # so the o_proj gather is a 2D-contiguous DMA per block.
            HSPLIT = max(1, (3 * NHL) // 4)
            HH = HSPLIT * HD
            HHB = (NHL - HSPLIT) * HD
            a2a_in_a = dram.tile([NC, HH, SQ], bf16)
            a2a_out_a = dram.tile([NC, HH, SQ], bf16)
            a2a_in_b = dram.tile([NC, HHB, SQ], bf16)
            a2a_out_b = dram.tile([NC, HHB, SQ], bf16)

            # ============ phase 1: projections + norm + rope ============
            with tc.tile_pool(name="wq_pool", bufs=1) as wq_pool, \
                 tc.tile_pool(name="xs_pool", bufs=3) as xs_pool, \
                 tc.tile_pool(name="p1sb", bufs=3) as p1sb, \
                 tc.tile_pool(name="p1small", bufs=6) as p1small, \
                 tc.tile_pool(name="pp_qg", bufs=3, space="PSUM") as pp_qg, \
                 tc.tile_pool(name="pp_kv", bufs=2, space="PSUM") as pp_kv, \
                 tc.tile_pool(name="pp_t1", bufs=3, space="PSUM") as pp_t1:

                wq_sb = wq_pool.tile([128, KO, NQG], bf16)
                wkv_sb = wq_pool.tile([128, KO, 2 * HD], bf16)
                # first chunks split fine so several DMA queues fill in
                # parallel and the first matmul starts early
                splits = [1, 1, 2, 4] + [4] * KO
                kq = 0
                while kq < KO:
                    KQ = min(splits.pop(0), KO - kq)
                    nc.sync.dma_start(
                        out=wq_sb[:, kq : kq + KQ, :],
                        in_=wq[:, kq * NQG : (kq + KQ) * NQG],
                    )
                    kq += KQ
                nc.sync.dma_start(
                    out=wkv_sb[:],
                    in_=wkv.rearrange("p (ko n) -> p ko n", ko=KO),
                )

                n_qg = (NQG + 511) // 512  # psum tiles per s-chunk (512 each)
                CW = KO * 128              # xT columns per s-chunk

                for i in range(SB):
                    xs = xs_pool.tile([128, KO, 128], bf16, tag="xs")
                    nc.sync.dma_start(
                        out=xs[:], in_=xT[:, i * CW : (i + 1) * CW]
                    )

                    # ---- projections into PSUM
                    qg_ps = []
                    for n2 in range(n_qg):
                        w = min(512, NQG - n2 * 512)
                        ps = pp_qg.tile([128, w], f32, tag="qg")
                        qg_ps.append(ps)
                        for ko in range(KO):
                            nc.tensor.matmul(
                                ps[:],
                                lhsT=xs[:, ko, :],
                                rhs=wq_sb[:, ko, n2 * 512 : n2 * 512 + w],
                                start=(ko == 0),
                                stop=(ko == KO - 1),
                            )
                    kv_ps = pp_kv.tile([128, 2 * HD], f32, tag="kv")
                    for ko in range(KO):
                        nc.tensor.matmul(
                            kv_ps[:],
                            lhsT=xs[:, ko, :],
                            rhs=wkv_sb[:, ko, :],
                            start=(ko == 0),
                            stop=(ko == KO - 1),
                        )

                    # ---- evict to one combined tile: [q(NHL*HD) | k | gate | v]
                    QW = NHL * HD
                    raw = p1sb.tile([128, NQG + 2 * HD], f32, tag="raw")
                    for n2, ps in enumerate(qg_ps):
                        lo = n2 * 512
                        hi = lo + ps.shape[1]
                        if lo < QW:           # q columns land at the same offset
                            e = min(hi, QW)
                            nc.vector.tensor_copy(
                                out=raw[:, lo:e], in_=ps[:, 0 : e - lo]
                            )
                        if hi > QW:           # gate columns shift right by HD
                            s0 = max(lo, QW)
                            nc.vector.tensor_copy(
                                out=raw[:, s0 + HD : hi + HD],
                                in_=ps[:, s0 - lo : hi - lo],
                            )
                    nc.vector.tensor_copy(
                        out=raw[:, QW : QW + HD], in_=kv_ps[:, 0:HD]
                    )
                    nc.vector.tensor_copy(
                        out=raw[:, NQG + HD : NQG + 2 * HD],
                        in_=kv_ps[:, HD : 2 * HD],
                    )
                    qk = raw[:, 0 : N5 * HD].rearrange(
                        "p (h d) -> p h d", h=N5
                    )                       # [128, 5, HD] q heads + k
                    gview = raw[
                        :, QW + HD : QW + HD + QW
                    ].rearrange("p (h d) -> p h d", h=NHL)

                    # ---- gates: raw stash (sigmoids batched at end of phase)
                    nc.vector.tensor_copy(out=gate_sb[:, i, :, :], in_=gview)

                    # ---- fused rmsnorm for q heads + k
                    sq5 = p1small.tile([128, N5, HD], f32, tag="sq5")
                    nc.vector.tensor_tensor(sq5[:], qk, qk, op=ALU.mult)
                    ssq5 = p1small.tile([128, N5], f32, tag="ssq5")
                    nc.vector.tensor_reduce(
                        ssq5[:], sq5[:], axis=mybir.AxisListType.X, op=ALU.add
                    )
                    rstd5 = p1small.tile([128, N5], f32, tag="rstd5")
                    nc.scalar.activation(
                        out=rstd5[:], in_=ssq5[:], func=AF.Sqrt,
                        scale=1.0 / HD, bias=eps_sb[:],
                    )
                    nc.vector.reciprocal(rstd5[:], rstd5[:])
                    qkn = p1sb.tile([128, N5, HD], f32, tag="qkn")
                    nc.vector.tensor_tensor(
                        qkn[:], qk,
                        rstd5[:, :, None].to_broadcast([128, N5, HD]),
                        op=ALU.mult,
                    )
                    nc.vector.tensor_tensor(qkn[:], qkn[:], w1qk_sb[:], op=ALU.mult)

                    # ---- rope into f32 rot + bf16 cast
                    cc = cos5_sb[:, i, :, :]
                    ss = sin5_sb[:, i, :, :]
                    x1 = qkn[:, :, 0:ROT:2]
                    x2 = qkn[:, :, 1:ROT:2]
                    rot = p1small.tile([128, N5, ROT], f32, tag="rot")
                    re = rot[:, :, 0:ROT:2]
                    ro = rot[:, :, 1:ROT:2]
                    t1 = p1small.tile([128, N5, RH], f32, tag="t1")
                    t2 = p1small.tile([128, N5, RH], f32, tag="t2")
                    nc.vector.tensor_tensor(t1[:], x2, ss, op=ALU.mult)
                    nc.vector.tensor_tensor(re, x1, cc, op=ALU.mult)
                    nc.vector.tensor_tensor(re, re, t1[:], op=ALU.subtract)
                    nc.vector.tensor_tensor(t2[:], x1, ss, op=ALU.mult)
                    nc.vector.tensor_tensor(ro, x2, cc, op=ALU.mult)
                    nc.vector.tensor_tensor(ro, ro, t2[:], op=ALU.add)

                    qk5b = p1sb.tile([128, N5, HD], bf16, tag="qk5b")
                    nc.vector.tensor_copy(out=qk5b[:, :, 0:ROT], in_=rot[:])
                    nc.vector.tensor_copy(
                        out=qk5b[:, :, ROT:HD], in_=qkn[:, :, ROT:HD]
                    )

                    # ---- k cache (f32: rotated part + untouched tail)
                    nc.gpsimd.dma_start(
                        out=k_cache[i * 128 : (i + 1) * 128, 0:ROT],
                        in_=rot[:, NHL, :],
                    )
                    nc.gpsimd.dma_start(
                        out=k_cache[i * 128 : (i + 1) * 128, ROT:HD],
                        in_=qkn[:, NHL, ROT:HD],
                    )

                    # ---- transposes into qT / kT
                    for h in range(N5):
                        tp = pp_t1.tile([128, 128], bf16, tag="tpb")
                        nc.tensor.transpose(tp[:], qk5b[:, h, :], ident_b[:])
                        dst = (
                            qT_sb[:, h, i * 128 : (i + 1) * 128]
                            if h < NHL
                            else kT_sb[:, i * 128 : (i + 1) * 128]
                        )
                        nc.vector.tensor_copy(out=dst, in_=tp[:])

                    # ---- v: bf16 stash + f32 cache
                    nc.vector.tensor_copy(
                        out=v_sb[:, i, 0:HD], in_=raw[:, NQG + HD : NQG + 2 * HD]
                    )
                    nc.gpsimd.dma_start(
                        out=v_cache[i * 128 : (i + 1) * 128, :],
                        in_=raw[:, NQG + HD : NQG + 2 * HD],
                    )

                # gates: back-to-back sigmoids (one ACT table load)
                for i in range(SB):
                    nc.scalar.activation(
                        out=gate_sb[:, i, :, :], in_=gate_sb[:, i, :, :],
                        func=AF.Sigmoid,
                    )

            # ============ phase 2: attention ============
            with tc.tile_pool(name="exp_pool", bufs=SB + 2) as exp_pool, \
                 tc.tile_pool(name="ag_pool", bufs=4) as ag_pool, \
                 tc.tile_pool(name="at_small", bufs=8) as at_small, \
                 tc.tile_pool(name="pp_s", bufs=4, space="PSUM") as pp_s, \
                 tc.tile_pool(name="pp_o", bufs=2, space="PSUM") as pp_o, \
                 tc.tile_pool(name="pp_t2", bufs=2, space="PSUM") as pp_t2:

                for h in range(NHL):
                    for j in range(NBAND):
                        ntb = TPB * (j + 1)        # t-blocks this band
                        exp_tiles = []             # (tile, global col start)
                        for tb in range(ntb):
                            s_lo = max(j * BAND, tb * 128)
                            ne = (j + 1) * BAND - s_lo
                            ps = pp_s.tile([128, BAND], f32, tag="ps")
                            nc.tensor.matmul(
                                ps[:, :ne],
                                lhsT=kT_sb[:, tb * 128 : (tb + 1) * 128],
                                rhs=qT_sb[:, h, s_lo : (j + 1) * BAND],
                                start=True, stop=True,
                            )
                            et = exp_pool.tile([128, BAND], bf16, tag="expT")
                            nc.scalar.activation(
                                out=et[:, :ne], in_=ps[:, :ne],
                                func=AF.Exp, scale=SCALE,
                            )
                            if tb * 128 >= j * BAND:   # diagonal block
                                nc.vector.tensor_tensor(
                                    et[:, 0:128], et[:, 0:128], tri_sb[:],
                                    op=ALU.mult,
                                )
                            exp_tiles.append((et, s_lo))

                        for sl in range(TPB):
                            sblk = j * TPB + sl      # global 128-row s block
                            po = pp_o.tile([128, HD + 1], f32, tag="po")
                            for tb in range(sblk + 1):
                                et, s_lo = exp_tiles[tb]
                                co = sblk * 128 - s_lo
                                nc.tensor.matmul(
                                    po[:],
                                    lhsT=et[:, co : co + 128],
                                    rhs=v_sb[:, tb, :],
                                    start=(tb == 0),
                                    stop=(tb == sblk),
                                )
                            rec = at_small.tile([128, 1], f32, tag="rec")
                            nc.vector.reciprocal(rec[:], po[:, HD : HD + 1])
                            ag = ag_pool.tile([128, HD], bf16, tag="ag")
                            nc.vector.tensor_scalar(
                                ag[:], po[:, 0:HD], rec[:], None, op0=ALU.mult
                            )
                            nc.vector.tensor_tensor(
                                ag[:], ag[:], gate_sb[:, sblk, h, :], op=ALU.mult
                            )
                            tp2 = pp_t2.tile([128, 128], bf16, tag="tp2")
                            nc.tensor.transpose(tp2[:], ag[:], ident_b[:])
                            agb = ag_pool.tile([128, 128], bf16, tag="agb")
                            nc.vector.tensor_copy(out=agb[:], in_=tp2[:])
                            # scatter into a2a shards; rows p*HSPLIT + h_half
                            in_a = h < HSPLIT
                            a2a_in = a2a_in_a if in_a else a2a_in_b
                            hw_ = HSPLIT if in_a else NHL - HSPLIT
                            hh = h if in_a else h - HSPLIT
                            s0 = sblk * 128
                            jlo, jhi = s0 // SQ, (s0 + 127) // SQ
                            for jj in range(jlo, jhi + 1):
                                lo = max(s0, jj * SQ)
                                hi = min(s0 + 128, (jj + 1) * SQ)
                                dst = a2a_in[jj].rearrange(
                                    "(p h) s -> p h s", h=hw_
                                )
                                nc.sync.dma_start(
                                    out=dst[:, hh, lo - jj * SQ : hi - jj * SQ],
                                    in_=agb[:, lo - s0 : hi - s0],
                                )
                    if h == HSPLIT - 1:
                        nc.gpsimd.collective_compute(
                            "AllToAll", ALU.bypass,
                            replica_groups=[list(range(NC))],
                            ins=[a2a_in_a[:].opt()],
                            outs=[a2a_out_a[:].opt()],
                        )
                nc.gpsimd.collective_compute(
                    "AllToAll", ALU.bypass,
                    replica_groups=[list(range(NC))],
                    ins=[a2a_in_b[:].opt()],
                    outs=[a2a_out_b[:].opt()],
                )

            # ============ phase 4: o_proj ============
            # Split contraction in head-halves: pass A (a2a half a) runs while
            # the second AllToAll is still in flight; pass B adds on top.
            SL = (SQ + 127) // 128
            PSL = min(128, SQ)
            GB = NC // 2                      # kv-group blocks per batch

            with tc.tile_pool(name="agf_pool", bufs=1) as agf_pool, \
                 tc.tile_pool(name="wo_pool", bufs=NOB) as wo_pool, \
                 tc.tile_pool(name="op_out", bufs=NOB * 2 * SL + 1) as op_out, \
                 tc.tile_pool(name="pp_op", bufs=4, space="PSUM") as pp_op:

                wo_tiles = []
                for nb in range(NOB):
                    nw = min(512, HID - nb * 512)
                    wo_nb = wo_pool.tile([128, KO2, 512], bf16, tag="wo_nb")
                    wo_tiles.append(wo_nb)
                    nc.sync.dma_start(
                        out=wo_nb[:, :, :nw],
                        in_=wo[
                            :, nb * KO2 * 512 : (nb + 1) * KO2 * 512
                        ].rearrange("p (ko n) -> p ko n", ko=KO2),
                    )

                # gathered [p, batch, blk, h_half, sq] per half
                agf_a = agf_pool.tile([128, 2, GB, HSPLIT, SQ], bf16)
                agf_b = agf_pool.tile([128, 2, GB, NHL - HSPLIT, SQ], bf16)
                for bb in range(2):
                    for blk in range(GB):
                        nc.sync.dma_start(
                            out=agf_a[:, bb, blk, :, :],
                            in_=a2a_out_a[bb * GB + blk].rearrange(
                                "(p h) s -> p h s", h=HSPLIT
                            ),
                        )
                        nc.sync.dma_start(
                            out=agf_b[:, bb, blk, :, :],
                            in_=a2a_out_b[bb * GB + blk].rearrange(
                                "(p h) s -> p h s", h=NHL - HSPLIT
                            ),
                        )

                def lhs_for(bb, k2, sl):
                    blk, h = divmod(k2, NHL)
                    buf = agf_a if h < HSPLIT else agf_b
                    hh = h if h < HSPLIT else h - HSPLIT
                    return buf[:, bb, blk, hh, sl * 128 : sl * 128 + PSL]

                k2_a = [k2 for k2 in range(KO2) if k2 % NHL < HSPLIT]
                k2_b = [k2 for k2 in range(KO2) if k2 % NHL >= HSPLIT]


                of_tiles = {}
                for nb in range(NOB):          # pass A (needs only half a)
                    nw = min(512, HID - nb * 512)
                    for bb in range(2):
                        for sl in range(SL):
                            pso = pp_op.tile([PSL, 512], f32, tag="pso")
                            for idx, k2 in enumerate(k2_a):
                                nc.tensor.matmul(
                                    pso[:, :nw],
                                    lhsT=lhs_for(bb, k2, sl),
                                    rhs=wo_tiles[nb][:, k2, :nw],
                                    start=(idx == 0),
                                    stop=(idx == len(k2_a) - 1),
                                )
                            of = op_out.tile([PSL, 512], f32, tag="of")
                            of_tiles[(nb, bb, sl)] = of
                            nc.vector.tensor_copy(out=of[:, :nw], in_=pso[:, :nw])
                for nb in range(NOB):          # pass B (adds half b)
                    nw = min(512, HID - nb * 512)
                    for bb in range(2):
                        for sl in range(SL):
                            pso = pp_op.tile([PSL, 512], f32, tag="pso")
                            for idx, k2 in enumerate(k2_b):
                                nc.tensor.matmul(
                                    pso[:, :nw],
                                    lhsT=lhs_for(bb, k2, sl),
                                    rhs=wo_tiles[nb][:, k2, :nw],
                                    start=(idx == 0),
                                    stop=(idx == len(k2_b) - 1),
                                )
                            of = of_tiles[(nb, bb, sl)]
                            nc.vector.tensor_tensor(
                                of[:, :nw], of[:, :nw], pso[:, :nw], op=ALU.add
                            )
                            nc.gpsimd.dma_start(
                                out=out[
                                    bb, sl * 128 : sl * 128 + PSL,
                                    nb * 512 : nb * 512 + nw,
                                ],
                                in_=of[:, :nw],
                            )

    return nc


# ---------------------------------------------------------------- host side

def _pack_rows(a, p=128):
    """[R, N] with R = k*p  ->  [p, k*N] grouping rows by (k, p)."""
    R, N = a.shape
    k = R // p
    return np.ascontiguousarray(a.reshape(k, p, N).transpose(1, 0, 2).reshape(p, k * N))


def _shard_inputs(inputs, cfg):
    """Build per-core in_maps from the full problem inputs."""
    c = _derived(cfg)
    B, S, HID, NH, HD, ROT = c["B"], c["S"], c["HID"], c["NH"], c["HD"], c["ROT"]
    GC, NHL, NC, KO, SB = c["GC"], c["NHL"], c["NCORES"], c["KO"], c["SB"]
    KO2, NOB = c["KO2"], c["NOB"]
    N5 = NHL + 1
    RH = ROT // 2
    bf = ml_dtypes.bfloat16

    hs = np.asarray(inputs["hidden_states"], np.float32)
    wq = np.asarray(inputs["wq"], np.float32)
    wk = np.asarray(inputs["wk"], np.float32)
    wv = np.asarray(inputs["wv"], np.float32)
    wo = np.asarray(inputs["wo"], np.float32)
    q_norm_w = np.asarray(inputs["q_norm_w"], np.float32)
    k_norm_w = np.asarray(inputs["k_norm_w"], np.float32)
    rope_cos = np.ascontiguousarray(np.asarray(inputs["rope_cos"], np.float32)[:S])
    rope_sin = np.ascontiguousarray(np.asarray(inputs["rope_sin"], np.float32)[:S])

    # wo packed per n-band: [p, nb, ko2, n']
    wo_b = (
        wo.astype(bf)
        .reshape(KO2, 128, NOB, 512)
        .transpose(1, 2, 0, 3)
        .reshape(128, NOB * KO2 * 512)
    )
    wo_b = np.ascontiguousarray(wo_b)

    w1qk = np.concatenate(
        [np.tile(1.0 + q_norm_w, NHL), 1.0 + k_norm_w]
    )[None, :].repeat(128, 0).copy()

    # rope tables packed [p, sb, h(=N5 copies), f]
    def pack_rope(t):
        r = t.reshape(SB, 128, RH).transpose(1, 0, 2)          # [p, sb, f]
        r = np.repeat(r[:, :, None, :], N5, axis=2)            # [p, sb, h, f]
        return np.ascontiguousarray(r.reshape(128, SB * N5 * RH))

    cos5 = pack_rope(rope_cos)
    sin5 = pack_rope(rope_sin)

    tt = np.arange(128)
    tri = (tt[None, :] >= tt[:, None]).astype(bf)

    in_maps = []
    for core in range(NC):
        b, g = divmod(core, GC)
        # x^T packed per s-chunk: [p, sb, ko, s_lo]
        xb = hs[b].T.astype(bf)                                # [HID, S]
        xTc = np.ascontiguousarray(
            xb.reshape(KO, 128, SB, 128)
            .transpose(1, 2, 0, 3)
            .reshape(128, SB * KO * 128)
        )
        # wq columns reordered to [q0..q3 | gate0..gate3], then row-packed
        wq_g = wq[:, g * NHL * 2 * HD : (g + 1) * NHL * 2 * HD]
        wq_r = wq_g.reshape(HID, NHL, 2, HD)
        wq_dev = np.concatenate(
            [wq_r[:, :, 0, :].reshape(HID, NHL * HD),
             wq_r[:, :, 1, :].reshape(HID, NHL * HD)], axis=1
        ).astype(bf)
        wkv_dev = np.concatenate(
            [wk[:, g * HD : (g + 1) * HD], wv[:, g * HD : (g + 1) * HD]],
            axis=1,
        ).astype(bf)
        in_maps.append(
            dict(
                xT=xTc,
                wq=_pack_rows(wq_dev),
                wkv=_pack_rows(wkv_dev),
                wo=wo_b,
                cos5=cos5, sin5=sin5, w1qk=w1qk, tri=tri,
            )
        )
    return in_maps


def assemble_outputs(results, cfg):
    c = _derived(cfg)
    B, S, HID, HD = c["B"], c["S"], c["HID"], c["HD"]
    GC, NC, SQ, NKV = c["GC"], c["NCORES"], c["SQ"], c["NKV"]

    out = np.empty((B, S, HID), np.float32)
    k_cache = np.empty((B, NKV, S, HD), np.float32)
    v_cache = np.empty((B, NKV, S, HD), np.float32)
    for core in range(NC):
        r = results[core]
        b, g = divmod(core, GC)
        out[:, core * SQ : (core + 1) * SQ, :] = np.asarray(r["out"]).reshape(
            B, SQ, HID
        )
        k_cache[b, g] = np.asarray(r["k_cache"]).reshape(S, HD)
        v_cache[b, g] = np.asarray(r["v_cache"]).reshape(S, HD)
    return out, k_cache, v_cache


_NC_CACHE = {}


def kernel(**inputs):
    from concourse.bass_utils import run_bass_kernel_spmd

    cfg = FULL_CFG
    key = "full"
    if key not in _NC_CACHE:
        nc = build_nc(cfg)
        nc.finalize()
        _NC_CACHE[key] = nc
    nc = _NC_CACHE[key]

    in_maps = _shard_inputs(inputs, cfg)
    trace = bool(int(os.environ.get("KERNEL_TRACE", "0")))
    res = run_bass_kernel_spmd(
        nc, in_maps, core_ids=list(range(_derived(cfg)["NCORES"])), trace=trace
    )
    if trace and res.exec_time_ns is not None:
        print(f"HW exec time: {res.exec_time_ns} ns", flush=True)
        kernel.last_exec_time_ns = res.exec_time_ns
    return assemble_outputs(res.results, cfg)


# revision 22
# speedup vs baseline: 1.0890x; 1.0890x over previous
"""Distributed Bass kernel for fused GQA attention block (ANEFullAttention).

Full op: qkv-proj (wq also produces a sigmoid gate), q/k rmsnorm, partial
interleaved RoPE (first 32 of 128 dims), causal GQA attention (16 q heads /
4 kv heads), gate multiply, o_proj; returns (out, k_cache, v_cache).

Sharding over 8 cores: core c -> (batch b = c//4, head-group g = c%4).
Each core owns 4 q heads + 1 kv head of one batch.  o_proj is handled by an
8-way AllToAll (split in two halves by head pair, so the first half overlaps
the remaining attention work): each core ships its gated attention output
(transposed, [d_local, S]) sliced into 8 s-shards; afterwards core j holds
the full 2048-dim attention output for BOTH batches on s-rows
[j*S/8,(j+1)*S/8) and computes that slice of o_proj against replicated wo.

Perf notes (measured on TRN2):
- All weight/activation DRAM parameters are pre-packed on the host into the
  exact [128, ...contiguous-free-dim] device layout so every HWDGE dma_start
  is a cheap 2D pattern (multi-dim APs are descriptor-generated inline on
  the issuing sequencer at ~5-12us per call).
- ScalarE(ACT) runs only Exp + batched Sigmoid/Sqrt (activation-table swaps
  cost ~1.3us); DVE does PSUM evictions, norm scaling, rope, masks, gating.
- q heads and the kv head share one fused norm/rope pipeline (wq columns are
  host-reordered to [q0..q3 | gate0..gate3] so q|k rows are contiguous).
- walrus runs with ldw-opt disabled: every matmul pays its own LDWEIGHTS,
  so matmul count is kept minimal and moving dims maximal.
"""

import os
import sys

_TRN_REPO = "/opt/trn_rl_repo"
if _TRN_REPO not in sys.path:
    sys.path.insert(0, _TRN_REPO)

import numpy as np
import ml_dtypes

# ---------------------------------------------------------------- config

FULL_CFG = dict(
    B=2, S=2048, HID=2048, NH=16, NKV=4, HD=128, ROT=32,
    THETA=10000000.0, EPS=1e-6,
)


def _derived(cfg):
    d = dict(cfg)
    d["GC"] = 4                       # head-groups (tensor-parallel degree)
    d["NCORES"] = 8
    d["NHL"] = d["NH"] // d["GC"]     # q heads per core
    d["KO"] = d["HID"] // 128         # contraction blocks for projections
    d["SB"] = d["S"] // 128           # 128-row s blocks
    d["BAND"] = min(512, d["S"])      # moving-dim width for score matmuls
    d["NBAND"] = d["S"] // d["BAND"]
    d["TPB"] = d["BAND"] // 128       # t-blocks per band
    d["SQ"] = d["S"] // d["NCORES"]   # per-core o_proj s-slice (per batch)
    d["DM"] = d["NH"] * d["HD"]       # attention model dim (o_proj contraction)
    d["KO2"] = d["DM"] // 128
    d["NOB"] = (d["HID"] + 511) // 512
    d["SCALE"] = d["HD"] ** -0.5
    return d


# ---------------------------------------------------------------- builder

def build_nc(cfg):
    import concourse.bass as bass
    import concourse.tile as tile
    import concourse.mybir as mybir
    from concourse import bacc
    from concourse.masks import make_identity

    c = _derived(cfg)
    S, HID, HD, ROT = c["S"], c["HID"], c["HD"], c["ROT"]
    NHL, KO, SB = c["NHL"], c["KO"], c["SB"]
    BAND, NBAND, TPB, SQ = c["BAND"], c["NBAND"], c["TPB"], c["SQ"]
    KO2, NOB, SCALE, EPS = c["KO2"], c["NOB"], c["SCALE"], c["EPS"]
    NC = c["NCORES"]
    NQG = NHL * 2 * HD                # 1024 (q heads then gates)
    N5 = NHL + 1                      # q heads + the kv head
    RH = ROT // 2

    f32 = mybir.dt.float32
    bf16 = mybir.dt.bfloat16
    AF = mybir.ActivationFunctionType
    ALU = mybir.AluOpType

    nc = bacc.Bacc(None, target_bir_lowering=False, debug=False, num_devices=NC)

    # -------- dram parameters (names = in_map keys; all pre-packed 2D)
    xT = nc.declare_dram_parameter("xT", [128, SB * KO * 128], bf16, isOutput=False)
    wq = nc.declare_dram_parameter("wq", [128, KO * NQG], bf16, isOutput=False)
    wkv = nc.declare_dram_parameter("wkv", [128, KO * 2 * HD], bf16, isOutput=False)
    wo = nc.declare_dram_parameter("wo", [128, NOB * KO2 * 512], bf16, isOutput=False)
    cos5 = nc.declare_dram_parameter("cos5", [128, SB * N5 * RH], f32, isOutput=False)
    sin5 = nc.declare_dram_parameter("sin5", [128, SB * N5 * RH], f32, isOutput=False)
    w1qk = nc.declare_dram_parameter("w1qk", [128, N5 * HD], f32, isOutput=False)
    tri = nc.declare_dram_parameter("tri", [128, 128], bf16, isOutput=False)

    out = nc.declare_dram_parameter("out", [2, SQ, HID], f32, isOutput=True)
    k_cache = nc.declare_dram_parameter("k_cache", [S, HD], f32, isOutput=True)
    v_cache = nc.declare_dram_parameter("v_cache", [S, HD], f32, isOutput=True)

    with tile.TileContext(nc) as tc:
        with tc.tile_pool(name="const", bufs=1) as const, \
             tc.tile_pool(name="persist", bufs=1) as persist, \
             tc.tile_pool(name="dram", bufs=1, space="DRAM") as dram:
            ident_b = const.tile([128, 128], bf16)
            make_identity(nc, ident_b)

            w1qk_sb = const.tile([128, N5, HD], f32)
            nc.gpsimd.dma_start(
                out=w1qk_sb[:], in_=w1qk.rearrange("p (h d) -> p h d", h=N5)
            )
            tri_sb = const.tile([128, 128], bf16)
            nc.gpsimd.dma_start(out=tri_sb[:], in_=tri[:, :])
            eps_sb = const.tile([128, 1], f32)
            nc.vector.memset(eps_sb[:], EPS)

            cos5_sb = const.tile([128, SB, N5, RH], f32)
            nc.gpsimd.dma_start(
                out=cos5_sb[:],
                in_=cos5.rearrange("p (sb h f) -> p sb h f", sb=SB, h=N5),
            )
            sin5_sb = const.tile([128, SB, N5, RH], f32)
            nc.gpsimd.dma_start(
                out=sin5_sb[:],
                in_=sin5.rearrange("p (sb h f) -> p sb h f", sb=SB, h=N5),
            )

            # phase-1 -> phase-2 tensors
            qT_sb = persist.tile([128, NHL, S], bf16)      # [d, h, s]
            kT_sb = persist.tile([128, S], bf16)           # [d, t]
            v_sb = persist.tile([128, SB, HD + 1], bf16)   # [t_lo, t_blk, d+ones]
            gate_sb = persist.tile([128, SB, NHL, HD], bf16)
            nc.vector.memset(v_sb[:, :, HD : HD + 1], 1.0)

            # a2a bounce buffers, split in half by head; rows (p*HSPLIT + h)
            # so the o_proj gather is a 2D-contiguous DMA per block.
            HSPLIT = max(1, NHL // 2)
            HH = HSPLIT * HD
            a2a_in_a = dram.tile([NC, HH, SQ], bf16)
            a2a_out_a = dram.tile([NC, HH, SQ], bf16)
            a2a_in_b = dram.tile([NC, HH, SQ], bf16)
            a2a_out_b = dram.tile([NC, HH, SQ], bf16)

            # ============ phase 1: projections + norm + rope ============
            with tc.tile_pool(name="wq_pool", bufs=1) as wq_pool, \
                 tc.tile_pool(name="xs_pool", bufs=3) as xs_pool, \
                 tc.tile_pool(name="p1sb", bufs=3) as p1sb, \
                 tc.tile_pool(name="p1small", bufs=6) as p1small, \
                 tc.tile_pool(name="pp_qg", bufs=3, space="PSUM") as pp_qg, \
                 tc.tile_pool(name="pp_kv", bufs=2, space="PSUM") as pp_kv, \
                 tc.tile_pool(name="pp_t1", bufs=3, space="PSUM") as pp_t1:

                wq_sb = wq_pool.tile([128, KO, NQG], bf16)
                wkv_sb = wq_pool.tile([128, KO, 2 * HD], bf16)
                # first chunks split fine so several DMA queues fill in
                # parallel and the first matmul starts early
                splits = [1, 1, 2, 4] + [4] * KO
                kq = 0
                while kq < KO:
                    KQ = min(splits.pop(0), KO - kq)
                    nc.sync.dma_start(
                        out=wq_sb[:, kq : kq + KQ, :],
                        in_=wq[:, kq * NQG : (kq + KQ) * NQG],
                    )
                    kq += KQ
                nc.sync.dma_start(
                    out=wkv_sb[:],
                    in_=wkv.rearrange("p (ko n) -> p ko n", ko=KO),
                )

                n_qg = (NQG + 511) // 512  # psum tiles per s-chunk (512 each)
                CW = KO * 128              # xT columns per s-chunk

                for i in range(SB):
                    xs = xs_pool.tile([128, KO, 128], bf16, tag="xs")
                    nc.sync.dma_start(
                        out=xs[:], in_=xT[:, i * CW : (i + 1) * CW]
                    )

                    # ---- projections into PSUM
                    qg_ps = []
                    for n2 in range(n_qg):
                        w = min(512, NQG - n2 * 512)
                        ps = pp_qg.tile([128, w], f32, tag="qg")
                        qg_ps.append(ps)
                        for ko in range(KO):
                            nc.tensor.matmul(
                                ps[:],
                                lhsT=xs[:, ko, :],
                                rhs=wq_sb[:, ko, n2 * 512 : n2 * 512 + w],
                                start=(ko == 0),
                                stop=(ko == KO - 1),
                            )
                    kv_ps = pp_kv.tile([128, 2 * HD], f32, tag="kv")
                    for ko in range(KO):
                        nc.tensor.matmul(
                            kv_ps[:],
                            lhsT=xs[:, ko, :],
                            rhs=wkv_sb[:, ko, :],
                            start=(ko == 0),
                            stop=(ko == KO - 1),
                        )

                    # ---- evict to one combined tile: [q(NHL*HD) | k | gate | v]
                    QW = NHL * HD
                    raw = p1sb.tile([128, NQG + 2 * HD], f32, tag="raw")
                    for n2, ps in enumerate(qg_ps):
                        lo = n2 * 512
                        hi = lo + ps.shape[1]
                        if lo < QW:           # q columns land at the same offset
                            e = min(hi, QW)
                            nc.vector.tensor_copy(
                                out=raw[:, lo:e], in_=ps[:, 0 : e - lo]
                            )
                        if hi > QW:           # gate columns shift right by HD
                            s0 = max(lo, QW)
                            nc.vector.tensor_copy(
                                out=raw[:, s0 + HD : hi + HD],
                                in_=ps[:, s0 - lo : hi - lo],
                            )
                    nc.vector.tensor_copy(
                        out=raw[:, QW : QW + HD], in_=kv_ps[:, 0:HD]
                    )
                    nc.vector.tensor_copy(
                        out=raw[:, NQG + HD : NQG + 2 * HD],
                        in_=kv_ps[:, HD : 2 * HD],
                    )
                    qk = raw[:, 0 : N5 * HD].rearrange(
                        "p (h d) -> p h d", h=N5
                    )                       # [128, 5, HD] q heads + k
                    gview = raw[
                        :, QW + HD : QW + HD + QW
                    ].rearrange("p (h d) -> p h d", h=NHL)

                    # ---- gates: raw stash (sigmoids batched at end of phase)
                    nc.vector.tensor_copy(out=gate_sb[:, i, :, :], in_=gview)

                    # ---- fused rmsnorm for q heads + k
                    sq5 = p1small.tile([128, N5, HD], f32, tag="sq5")
                    nc.vector.tensor_tensor(sq5[:], qk, qk, op=ALU.mult)
                    ssq5 = p1small.tile([128, N5], f32, tag="ssq5")
                    nc.vector.tensor_reduce(
                        ssq5[:], sq5[:], axis=mybir.AxisListType.X, op=ALU.add
                    )
                    rstd5 = p1small.tile([128, N5], f32, tag="rstd5")
                    nc.scalar.activation(
                        out=rstd5[:], in_=ssq5[:], func=AF.Sqrt,
                        scale=1.0 / HD, bias=eps_sb[:],
                    )
                    nc.vector.reciprocal(rstd5[:], rstd5[:])
                    qkn = p1sb.tile([128, N5, HD], f32, tag="qkn")
                    nc.vector.tensor_tensor(
                        qkn[:], qk,
                        rstd5[:, :, None].to_broadcast([128, N5, HD]),
                        op=ALU.mult,
                    )
                    nc.vector.tensor_tensor(qkn[:], qkn[:], w1qk_sb[:], op=ALU.mult)

                    # ---- rope into f32 rot + bf16 cast
                    cc = cos5_sb[:, i, :, :]
                    ss = sin5_sb[:, i, :, :]
                    x1 = qkn[:, :, 0:ROT:2]
                    x2 = qkn[:, :, 1:ROT:2]
                    rot = p1small.tile([128, N5, ROT], f32, tag="rot")
                    re = rot[:, :, 0:ROT:2]
                    ro = rot[:, :, 1:ROT:2]
                    t1 = p1small.tile([128, N5, RH], f32, tag="t1")
                    t2 = p1small.tile([128, N5, RH], f32, tag="t2")
                    nc.vector.tensor_tensor(t1[:], x2, ss, op=ALU.mult)
                    nc.vector.tensor_tensor(re, x1, cc, op=ALU.mult)
                    nc.vector.tensor_tensor(re, re, t1[:], op=ALU.subtract)
                    nc.vector.tensor_tensor(t2[:], x1, ss, op=ALU.mult)
                    nc.vector.tensor_tensor(ro, x2, cc, op=ALU.mult)
                    nc.vector.tensor_tensor(ro, ro, t2[:], op=ALU.add)

                    qk5b = p1sb.tile([128, N5, HD], bf16, tag="qk5b")
                    nc.vector.tensor_copy(out=qk5b[:, :, 0:ROT], in_=rot[:])
                    nc.vector.tensor_copy(
                        out=qk5b[:, :, ROT:HD], in_=qkn[:, :, ROT:HD]
                    )

                    # ---- k cache (f32: rotated part + untouched tail)
                    nc.gpsimd.dma_start(
                        out=k_cache[i * 128 : (i + 1) * 128, 0:ROT],
                        in_=rot[:, NHL, :],
                    )
                    nc.gpsimd.dma_start(
                        out=k_cache[i * 128 : (i + 1) * 128, ROT:HD],
                        in_=qkn[:, NHL, ROT:HD],
                    )

                    # ---- transposes into qT / kT
                    for h in range(N5):
                        tp = pp_t1.tile([128, 128], bf16, tag="tpb")
                        nc.tensor.transpose(tp[:], qk5b[:, h, :], ident_b[:])
                        dst = (
                            qT_sb[:, h, i * 128 : (i + 1) * 128]
                            if h < NHL
                            else kT_sb[:, i * 128 : (i + 1) * 128]
                        )
                        nc.vector.tensor_copy(out=dst, in_=tp[:])

                    # ---- v: bf16 stash + f32 cache
                    nc.vector.tensor_copy(
                        out=v_sb[:, i, 0:HD], in_=raw[:, NQG + HD : NQG + 2 * HD]
                    )
                    nc.gpsimd.dma_start(
                        out=v_cache[i * 128 : (i + 1) * 128, :],
                        in_=raw[:, NQG + HD : NQG + 2 * HD],
                    )

                # gates: back-to-back sigmoids (one ACT table load)
                for i in range(SB):
                    nc.scalar.activation(
                        out=gate_sb[:, i, :, :], in_=gate_sb[:, i, :, :],
                        func=AF.Sigmoid,
                    )

            # ============ phase 2: attention ============
            with tc.tile_pool(name="exp_pool", bufs=SB + 2) as exp_pool, \
                 tc.tile_pool(name="ag_pool", bufs=4) as ag_pool, \
                 tc.tile_pool(name="at_small", bufs=8) as at_small, \
                 tc.tile_pool(name="pp_s", bufs=4, space="PSUM") as pp_s, \
                 tc.tile_pool(name="pp_o", bufs=2, space="PSUM") as pp_o, \
                 tc.tile_pool(name="pp_t2", bufs=2, space="PSUM") as pp_t2:

                for h in range(NHL):
                    for j in range(NBAND):
                        ntb = TPB * (j + 1)        # t-blocks this band
                        exp_tiles = []             # (tile, global col start)
                        for tb in range(ntb):
                            s_lo = max(j * BAND, tb * 128)
                            ne = (j + 1) * BAND - s_lo
                            ps = pp_s.tile([128, BAND], f32, tag="ps")
                            nc.tensor.matmul(
                                ps[:, :ne],
                                lhsT=kT_sb[:, tb * 128 : (tb + 1) * 128],
                                rhs=qT_sb[:, h, s_lo : (j + 1) * BAND],
                                start=True, stop=True,
                            )
                            et = exp_pool.tile([128, BAND], bf16, tag="expT")
                            nc.scalar.activation(
                                out=et[:, :ne], in_=ps[:, :ne],
                                func=AF.Exp, scale=SCALE,
                            )
                            if tb * 128 >= j * BAND:   # diagonal block
                                nc.vector.tensor_tensor(
                                    et[:, 0:128], et[:, 0:128], tri_sb[:],
                                    op=ALU.mult,
                                )
                            exp_tiles.append((et, s_lo))

                        for sl in range(TPB):
                            sblk = j * TPB + sl      # global 128-row s block
                            po = pp_o.tile([128, HD + 1], f32, tag="po")
                            for tb in range(sblk + 1):
                                et, s_lo = exp_tiles[tb]
                                co = sblk * 128 - s_lo
                                nc.tensor.matmul(
                                    po[:],
                                    lhsT=et[:, co : co + 128],
                                    rhs=v_sb[:, tb, :],
                                    start=(tb == 0),
                                    stop=(tb == sblk),
                                )
                            rec = at_small.tile([128, 1], f32, tag="rec")
                            nc.vector.reciprocal(rec[:], po[:, HD : HD + 1])
                            ag = ag_pool.tile([128, HD], bf16, tag="ag")
                            nc.vector.tensor_scalar(
                                ag[:], po[:, 0:HD], rec[:], None, op0=ALU.mult
                            )
                            nc.vector.tensor_tensor(
                                ag[:], ag[:], gate_sb[:, sblk, h, :], op=ALU.mult
                            )
                            tp2 = pp_t2.tile([128, 128], bf16, tag="tp2")
                            nc.tensor.transpose(tp2[:], ag[:], ident_b[:])
                            agb = ag_pool.tile([128, 128], bf16, tag="agb")
                            nc.vector.tensor_copy(out=agb[:], in_=tp2[:])
                            # scatter into a2a shards; rows p*HSPLIT + h_half
                            a2a_in = a2a_in_a if h < HSPLIT else a2a_in_b
                            hh = h % HSPLIT
                            s0 = sblk * 128
                            jlo, jhi = s0 // SQ, (s0 + 127) // SQ
                            for jj in range(jlo, jhi + 1):
                                lo = max(s0, jj * SQ)
                                hi = min(s0 + 128, (jj + 1) * SQ)
                                dst = a2a_in[jj].rearrange(
                                    "(p h) s -> p h s", h=HSPLIT
                                )
                                nc.sync.dma_start(
                                    out=dst[:, hh, lo - jj * SQ : hi - jj * SQ],
                                    in_=agb[:, lo - s0 : hi - s0],
                                )
                    if h == HSPLIT - 1:
                        nc.gpsimd.collective_compute(
                            "AllToAll", ALU.bypass,
                            replica_groups=[list(range(NC))],
                            ins=[a2a_in_a[:].opt()],
                            outs=[a2a_out_a[:].opt()],
                        )
                nc.gpsimd.collective_compute(
                    "AllToAll", ALU.bypass,
                    replica_groups=[list(range(NC))],
                    ins=[a2a_in_b[:].opt()],
                    outs=[a2a_out_b[:].opt()],
                )

            # ============ phase 4: o_proj ============
            # Split contraction in head-halves: pass A (a2a half a) runs while
            # the second AllToAll is still in flight; pass B adds on top.
            SL = (SQ + 127) // 128
            PSL = min(128, SQ)
            GB = NC // 2                      # kv-group blocks per batch

            with tc.tile_pool(name="agf_pool", bufs=1) as agf_pool, \
                 tc.tile_pool(name="wo_pool", bufs=NOB) as wo_pool, \
                 tc.tile_pool(name="op_out", bufs=NOB * 2 * SL + 1) as op_out, \
                 tc.tile_pool(name="pp_op", bufs=4, space="PSUM") as pp_op:

                wo_tiles = []
                for nb in range(NOB):
                    nw = min(512, HID - nb * 512)
                    wo_nb = wo_pool.tile([128, KO2, 512], bf16, tag="wo_nb")
                    wo_tiles.append(wo_nb)
                    nc.sync.dma_start(
                        out=wo_nb[:, :, :nw],
                        in_=wo[
                            :, nb * KO2 * 512 : (nb + 1) * KO2 * 512
                        ].rearrange("p (ko n) -> p ko n", ko=KO2),
                    )

                # gathered [p, batch, blk, h_half, sq] per half
                agf_a = agf_pool.tile([128, 2, GB, HSPLIT, SQ], bf16)
                agf_b = agf_pool.tile([128, 2, GB, NHL - HSPLIT, SQ], bf16)
                for bb in range(2):
                    for blk in range(GB):
                        nc.sync.dma_start(
                            out=agf_a[:, bb, blk, :, :],
                            in_=a2a_out_a[bb * GB + blk].rearrange(
                                "(p h) s -> p h s", h=HSPLIT
                            ),
                        )
                        nc.sync.dma_start(
                            out=agf_b[:, bb, blk, :, :],
                            in_=a2a_out_b[bb * GB + blk].rearrange(
                                "(p h) s -> p h s", h=NHL - HSPLIT
                            ),
                        )

                def lhs_for(bb, k2, sl):
                    blk, h = divmod(k2, NHL)
                    buf = agf_a if h < HSPLIT else agf_b
                    hh = h if h < HSPLIT else h - HSPLIT
                    return buf[:, bb, blk, hh, sl * 128 : sl * 128 + PSL]

                k2_a = [k2 for k2 in range(KO2) if k2 % NHL < HSPLIT]
                k2_b = [k2 for k2 in range(KO2) if k2 % NHL >= HSPLIT]


                of_tiles = {}
                for nb in range(NOB):          # pass A (needs only half a)
                    nw = min(512, HID - nb * 512)
                    for bb in range(2):
                        for sl in range(SL):
                            pso = pp_op.tile([PSL, 512], f32, tag="pso")
                            for idx, k2 in enumerate(k2_a):
                                nc.tensor.matmul(
                                    pso[:, :nw],
                                    lhsT=lhs_for(bb, k2, sl),
                                    rhs=wo_tiles[nb][:, k2, :nw],
                                    start=(idx == 0),
                                    stop=(idx == len(k2_a) - 1),
                                )
                            of = op_out.tile([PSL, 512], f32, tag="of")
                            of_tiles[(nb, bb, sl)] = of
                            nc.vector.tensor_copy(out=of[:, :nw], in_=pso[:, :nw])
                for nb in range(NOB):          # pass B (adds half b)
                    nw = min(512, HID - nb * 512)
                    for bb in range(2):
                        for sl in range(SL):
                            pso = pp_op.tile([PSL, 512], f32, tag="pso")
                            for idx, k2 in enumerate(k2_b):
                                nc.tensor.matmul(
                                    pso[:, :nw],
                                    lhsT=lhs_for(bb, k2, sl),
                                    rhs=wo_tiles[nb][:, k2, :nw],
                                    start=(idx == 0),
                                    stop=(idx == len(k2_b) - 1),
                                )
                            of = of_tiles[(nb, bb, sl)]
                            nc.vector.tensor_tensor(
                                of[:, :nw], of[:, :nw], pso[:, :nw], op=ALU.add
                            )
                            nc.gpsimd.dma_start(
                                out=out[
                                    bb, sl * 128 : sl * 128 + PSL,
                                    nb * 512 : nb * 512 + nw,
                                ],
                                in_=of[:, :nw],
                            )

    return nc


# ---------------------------------------------------------------- host side

def _pack_rows(a, p=128):
    """[R, N] with R = k*p  ->  [p, k*N] grouping rows by (k, p)."""
    R, N = a.shape
    k = R // p
    return np.ascontiguousarray(a.reshape(k, p, N).transpose(1, 0, 2).reshape(p, k * N))


def _shard_inputs(inputs, cfg):
    """Build per-core in_maps from the full problem inputs."""
    c = _derived(cfg)
    B, S, HID, NH, HD, ROT = c["B"], c["S"], c["HID"], c["NH"], c["HD"], c["ROT"]
    GC, NHL, NC, KO, SB = c["GC"], c["NHL"], c["NCORES"], c["KO"], c["SB"]
    KO2, NOB = c["KO2"], c["NOB"]
    N5 = NHL + 1
    RH = ROT // 2
    bf = ml_dtypes.bfloat16

    hs = np.asarray(inputs["hidden_states"], np.float32)
    wq = np.asarray(inputs["wq"], np.float32)
    wk = np.asarray(inputs["wk"], np.float32)
    wv = np.asarray(inputs["wv"], np.float32)
    wo = np.asarray(inputs["wo"], np.float32)
    q_norm_w = np.asarray(inputs["q_norm_w"], np.float32)
    k_norm_w = np.asarray(inputs["k_norm_w"], np.float32)
    rope_cos = np.ascontiguousarray(np.asarray(inputs["rope_cos"], np.float32)[:S])
    rope_sin = np.ascontiguousarray(np.asarray(inputs["rope_sin"], np.float32)[:S])

    # wo packed per n-band: [p, nb, ko2, n']
    wo_b = (
        wo.astype(bf)
        .reshape(KO2, 128, NOB, 512)
        .transpose(1, 2, 0, 3)
        .reshape(128, NOB * KO2 * 512)
    )
    wo_b = np.ascontiguousarray(wo_b)

    w1qk = np.concatenate(
        [np.tile(1.0 + q_norm_w, NHL), 1.0 + k_norm_w]
    )[None, :].repeat(128, 0).copy()

    # rope tables packed [p, sb, h(=N5 copies), f]
    def pack_rope(t):
        r = t.reshape(SB, 128, RH).transpose(1, 0, 2)          # [p, sb, f]
        r = np.repeat(r[:, :, None, :], N5, axis=2)            # [p, sb, h, f]
        return np.ascontiguousarray(r.reshape(128, SB * N5 * RH))

    cos5 = pack_rope(rope_cos)
    sin5 = pack_rope(rope_sin)

    tt = np.arange(128)
    tri = (tt[None, :] >= tt[:, None]).astype(bf)

    in_maps = []
    for core in range(NC):
        b, g = divmod(core, GC)
        # x^T packed per s-chunk: [p, sb, ko, s_lo]
        xb = hs[b].T.astype(bf)                                # [HID, S]
        xTc = np.ascontiguousarray(
            xb.reshape(KO, 128, SB, 128)
            .transpose(1, 2, 0, 3)
            .reshape(128, SB * KO * 128)
        )
        # wq columns reordered to [q0..q3 | gate0..gate3], then row-packed
        wq_g = wq[:, g * NHL * 2 * HD : (g + 1) * NHL * 2 * HD]
        wq_r = wq_g.reshape(HID, NHL, 2, HD)
        wq_dev = np.concatenate(
            [wq_r[:, :, 0, :].reshape(HID, NHL * HD),
             wq_r[:, :, 1, :].reshape(HID, NHL * HD)], axis=1
        ).astype(bf)
        wkv_dev = np.concatenate(
            [wk[:, g * HD : (g + 1) * HD], wv[:, g * HD : (g + 1) * HD]],
            axis=1,
        ).astype(bf)
        in_maps.append(
            dict(
                xT=xTc,
                wq=_pack_rows(wq_dev),
                wkv=_pack_rows(wkv_dev),
                wo=wo_b,
                cos5=cos5, sin5=sin5, w1qk=w1qk, tri=tri,
            )
        )
    return in_maps


def assemble_outputs(results, cfg):
    c = _derived(cfg)
    B, S, HID, HD = c["B"], c["S"], c["HID"], c["HD"]
    GC, NC, SQ, NKV = c["GC"], c["NCORES"], c["SQ"], c["NKV"]

    out = np.empty((B, S, HID), np.float32)
    k_cache = np.empty((B, NKV, S, HD), np.float32)
    v_cache = np.empty((B, NKV, S, HD), np.float32)
    for core in range(NC):
        r = results[core]
        b, g = divmod(core, GC)
        out[:, core * SQ : (core + 1) * SQ, :] = np.asarray(r["out"]).reshape(
            B, SQ, HID
        )
        k_cache[b, g] = np.asarray(r["k_cache"]).reshape(S, HD)
        v_cache[b, g] = np.asarray(r["v_cache"]).reshape(S, HD)
    return out, k_cache, v_cache


_NC_CACHE = {}


def kernel(**inputs):
    from concourse.bass_utils import run_bass_kernel_spmd

    cfg = FULL_CFG
    key = "full"
    if key not in _NC_CACHE:
        nc = build_nc(cfg)
        nc.finalize()
        _NC_CACHE[key] = nc
    nc = _NC_CACHE[key]

    in_maps = _shard_inputs(inputs, cfg)
    trace = bool(int(os.environ.get("KERNEL_TRACE", "0")))
    res = run_bass_kernel_spmd(
        nc, in_maps, core_ids=list(range(_derived(cfg)["NCORES"])), trace=trace
    )
    if trace and res.exec_time_ns is not None:
        print(f"HW exec time: {res.exec_time_ns} ns", flush=True)
        kernel.last_exec_time_ns = res.exec_time_ns
    return assemble_outputs(res.results, cfg)


# revision 29
# speedup vs baseline: 1.1273x; 1.0352x over previous
"""Distributed Bass kernel for fused GQA attention block (ANEFullAttention).

Full op: qkv-proj (wq also produces a sigmoid gate), q/k rmsnorm, partial
interleaved RoPE (first 32 of 128 dims), causal GQA attention (16 q heads /
4 kv heads), gate multiply, o_proj; returns (out, k_cache, v_cache).

Sharding over 8 cores: core c -> (batch b = c//4, head-group g = c%4).
Each core owns 4 q heads + 1 kv head of one batch.  o_proj is handled by an
8-way AllToAll (split in two halves by head pair, so the first half overlaps
the remaining attention work): each core ships its gated attention output
(transposed, [d_local, S]) sliced into 8 s-shards; afterwards core j holds
the full 2048-dim attention output for BOTH batches on s-rows
[j*S/8,(j+1)*S/8) and computes that slice of o_proj against replicated wo.

Perf notes (measured on TRN2):
- All weight/activation DRAM parameters are pre-packed on the host into the
  exact [128, ...contiguous-free-dim] device layout so every HWDGE dma_start
  is a cheap 2D pattern (multi-dim APs are descriptor-generated inline on
  the issuing sequencer at ~5-12us per call).
- ScalarE(ACT) runs only Exp + batched Sigmoid/Sqrt (activation-table swaps
  cost ~1.3us); DVE does PSUM evictions, norm scaling, rope, masks, gating.
- q heads and the kv head share one fused norm/rope pipeline (wq columns are
  host-reordered to [q0..q3 | gate0..gate3] so q|k rows are contiguous).
- walrus runs with ldw-opt disabled: every matmul pays its own LDWEIGHTS,
  so matmul count is kept minimal and moving dims maximal.
"""

import os
import sys

_TRN_REPO = "/opt/trn_rl_repo"
if _TRN_REPO not in sys.path:
    sys.path.insert(0, _TRN_REPO)

import numpy as np
import ml_dtypes

# ---------------------------------------------------------------- config

FULL_CFG = dict(
    B=2, S=2048, HID=2048, NH=16, NKV=4, HD=128, ROT=32,
    THETA=10000000.0, EPS=1e-6,
)


def _derived(cfg):
    d = dict(cfg)
    d["GC"] = 4                       # head-groups (tensor-parallel degree)
    d["NCORES"] = 8
    d["NHL"] = d["NH"] // d["GC"]     # q heads per core
    d["KO"] = d["HID"] // 128         # contraction blocks for projections
    d["SB"] = d["S"] // 128           # 128-row s blocks
    d["BAND"] = min(512, d["S"])      # moving-dim width for score matmuls
    d["NBAND"] = d["S"] // d["BAND"]
    d["TPB"] = d["BAND"] // 128       # t-blocks per band
    d["SQ"] = d["S"] // d["NCORES"]   # per-core o_proj s-slice (per batch)
    d["DM"] = d["NH"] * d["HD"]       # attention model dim (o_proj contraction)
    d["KO2"] = d["DM"] // 128
    d["NOB"] = (d["HID"] + 511) // 512
    d["SCALE"] = d["HD"] ** -0.5
    return d


# ---------------------------------------------------------------- builder

def build_nc(cfg):
    import concourse.bass as bass
    import concourse.tile as tile
    import concourse.mybir as mybir
    from concourse import bacc
    from concourse.masks import make_identity

    c = _derived(cfg)
    S, HID, HD, ROT = c["S"], c["HID"], c["HD"], c["ROT"]
    NHL, KO, SB = c["NHL"], c["KO"], c["SB"]
    BAND, NBAND, TPB, SQ = c["BAND"], c["NBAND"], c["TPB"], c["SQ"]
    KO2, NOB, SCALE, EPS = c["KO2"], c["NOB"], c["SCALE"], c["EPS"]
    NC = c["NCORES"]
    NQG = NHL * 2 * HD                # 1024 (q heads then gates)
    N5 = NHL + 1                      # q heads + the kv head
    RH = ROT // 2

    f32 = mybir.dt.float32
    bf16 = mybir.dt.bfloat16
    AF = mybir.ActivationFunctionType
    ALU = mybir.AluOpType

    nc = bacc.Bacc(None, target_bir_lowering=False, debug=False, num_devices=NC)

    # -------- dram parameters (names = in_map keys; all pre-packed 2D)
    xT = nc.declare_dram_parameter("xT", [128, SB * KO * 128], bf16, isOutput=False)
    wq = nc.declare_dram_parameter("wq", [128, KO * NQG], bf16, isOutput=False)
    wkv = nc.declare_dram_parameter("wkv", [128, KO * 2 * HD], bf16, isOutput=False)
    wo = nc.declare_dram_parameter("wo", [128, NOB * KO2 * 512], bf16, isOutput=False)
    cos5 = nc.declare_dram_parameter("cos5", [128, SB * N5 * RH], f32, isOutput=False)
    sin5 = nc.declare_dram_parameter("sin5", [128, SB * N5 * RH], f32, isOutput=False)
    w1qk = nc.declare_dram_parameter("w1qk", [128, N5 * HD], f32, isOutput=False)
    tri = nc.declare_dram_parameter("tri", [128, 128], bf16, isOutput=False)

    out = nc.declare_dram_parameter("out", [2, SQ, HID], f32, isOutput=True)
    k_cache = nc.declare_dram_parameter("k_cache", [S, HD], f32, isOutput=True)
    v_cache = nc.declare_dram_parameter("v_cache", [S, HD], f32, isOutput=True)

    with tile.TileContext(nc) as tc:
        with tc.tile_pool(name="const", bufs=1) as const, \
             tc.tile_pool(name="persist", bufs=1) as persist, \
             tc.tile_pool(name="dram", bufs=1, space="DRAM") as dram:
            ident_b = const.tile([128, 128], bf16)
            make_identity(nc, ident_b)

            w1qk_sb = const.tile([128, N5, HD], f32)
            nc.gpsimd.dma_start(
                out=w1qk_sb[:], in_=w1qk.rearrange("p (h d) -> p h d", h=N5)
            )
            tri_sb = const.tile([128, 128], bf16)
            nc.gpsimd.dma_start(out=tri_sb[:], in_=tri[:, :])
            eps_sb = const.tile([128, 1], f32)
            nc.vector.memset(eps_sb[:], EPS)

            cos5_sb = const.tile([128, SB, N5, RH], f32)
            nc.gpsimd.dma_start(
                out=cos5_sb[:],
                in_=cos5.rearrange("p (sb h f) -> p sb h f", sb=SB, h=N5),
            )
            sin5_sb = const.tile([128, SB, N5, RH], f32)
            nc.gpsimd.dma_start(
                out=sin5_sb[:],
                in_=sin5.rearrange("p (sb h f) -> p sb h f", sb=SB, h=N5),
            )

            # phase-1 -> phase-2 tensors
            qT_sb = persist.tile([128, NHL, S], bf16)      # [d, h, s]
            kT_sb = persist.tile([128, S], bf16)           # [d, t]
            v_sb = persist.tile([128, SB, HD + 1], bf16)   # [t_lo, t_blk, d+ones]
            gate_sb = persist.tile([128, SB, NHL, HD], bf16)
            nc.vector.memset(v_sb[:, :, HD : HD + 1], 1.0)

            # a2a bounce buffers, split in half by head; rows (p*HSPLIT + h)
            # so the o_proj gather is a 2D-contiguous DMA per block.
            HSPLIT = max(1, NHL // 2)
            HH = HSPLIT * HD
            a2a_in_a = dram.tile([NC, HH, SQ], bf16)
            a2a_out_a = dram.tile([NC, HH, SQ], bf16)
            a2a_in_b = dram.tile([NC, HH, SQ], bf16)
            a2a_out_b = dram.tile([NC, HH, SQ], bf16)

            # ============ phases 1+2 interleaved =========================
            # Attention band j only needs the first TPB*(j+1) s-chunks, so
            # projection chunk-groups and attention bands share one TensorE
            # stream: while TensorE runs a chunk group's matmuls, ScalarE
            # drains the previous band's Exp backlog (attention alone is
            # ACT-bound).  Within a band, heads are software-pipelined depth
            # 1: scores(h+1) are issued before attn@v(h) so attn@v never
            # waits on a cold Exp.  All PSUM pools coexist: acc 3 (qg0, qg1,
            # kv share one tag) + scores 2 + attn-out 2 + transpose 1 = 8.
            with tc.tile_pool(name="wq_pool", bufs=1) as wq_pool, \
                 tc.tile_pool(name="xs_pool", bufs=4) as xs_pool, \
                 tc.tile_pool(name="p1sb", bufs=4) as p1sb, \
                 tc.tile_pool(name="p1small", bufs=6) as p1small, \
                 tc.tile_pool(name="exp_pool", bufs=2 * SB + 2) as exp_pool, \
                 tc.tile_pool(name="ag_pool", bufs=4) as ag_pool, \
                 tc.tile_pool(name="at_small", bufs=8) as at_small, \
                 tc.tile_pool(name="pp_a", bufs=3, space="PSUM") as pp_a, \
                 tc.tile_pool(name="pp_s", bufs=2, space="PSUM") as pp_s, \
                 tc.tile_pool(name="pp_o", bufs=2, space="PSUM") as pp_o, \
                 tc.tile_pool(name="pp_t", bufs=1, space="PSUM") as pp_t:

                wq_sb = wq_pool.tile([128, KO, NQG], bf16)
                wkv_sb = wq_pool.tile([128, KO, 2 * HD], bf16)
                # first chunks split fine so several DMA queues fill in
                # parallel and the first matmul starts early
                splits = [1, 1, 2, 4] + [4] * KO
                kq = 0
                while kq < KO:
                    KQ = min(splits.pop(0), KO - kq)
                    nc.scalar.dma_start(
                        out=wq_sb[:, kq : kq + KQ, :],
                        in_=wq[:, kq * NQG : (kq + KQ) * NQG],
                    )
                    kq += KQ
                nc.scalar.dma_start(
                    out=wkv_sb[:],
                    in_=wkv.rearrange("p (ko n) -> p ko n", ko=KO),
                )

                n_qg = (NQG + 511) // 512  # psum tiles per s-chunk (512 each)
                CW = KO * 128              # xT columns per s-chunk

                def do_chunk(i):
                    xs = xs_pool.tile([128, KO, 128], bf16, tag="xs")
                    kh = KO // 2
                    nc.sync.dma_start(
                        out=xs[:, 0:kh, :],
                        in_=xT[:, i * CW : i * CW + CW // 2],
                    )
                    nc.sync.dma_start(
                        out=xs[:, kh:KO, :],
                        in_=xT[:, i * CW + CW // 2 : (i + 1) * CW],
                    )

                    qg_ps = []
                    for n2 in range(n_qg):
                        w = min(512, NQG - n2 * 512)
                        ps = pp_a.tile([128, w], f32, tag="acc",
                                       padded_shape=[128, 512])
                        qg_ps.append(ps)
                        for ko in range(KO):
                            nc.tensor.matmul(
                                ps[:],
                                lhsT=xs[:, ko, :],
                                rhs=wq_sb[:, ko, n2 * 512 : n2 * 512 + w],
                                start=(ko == 0),
                                stop=(ko == KO - 1),
                            )
                    kv_ps = pp_a.tile([128, 2 * HD], f32, tag="acc",
                                      padded_shape=[128, 512])
                    for ko in range(KO):
                        nc.tensor.matmul(
                            kv_ps[:],
                            lhsT=xs[:, ko, :],
                            rhs=wkv_sb[:, ko, :],
                            start=(ko == 0),
                            stop=(ko == KO - 1),
                        )

                    # evict to one combined tile: [q(NHL*HD) | k | gate | v]
                    QW = NHL * HD
                    raw = p1sb.tile([128, NQG + 2 * HD], f32, tag="raw")
                    for n2, ps in enumerate(qg_ps):
                        lo = n2 * 512
                        hi = lo + ps.shape[1]
                        if lo < QW:
                            e = min(hi, QW)
                            nc.vector.tensor_copy(
                                out=raw[:, lo:e], in_=ps[:, 0 : e - lo]
                            )
                        if hi > QW:
                            s0 = max(lo, QW)
                            nc.vector.tensor_copy(
                                out=raw[:, s0 + HD : hi + HD],
                                in_=ps[:, s0 - lo : hi - lo],
                            )
                    nc.vector.tensor_copy(
                        out=raw[:, QW : QW + HD], in_=kv_ps[:, 0:HD]
                    )
                    nc.vector.tensor_copy(
                        out=raw[:, NQG + HD : NQG + 2 * HD],
                        in_=kv_ps[:, HD : 2 * HD],
                    )
                    qk = raw[:, 0 : N5 * HD].rearrange(
                        "p (h d) -> p h d", h=N5
                    )
                    gview = raw[
                        :, QW + HD : QW + HD + QW
                    ].rearrange("p (h d) -> p h d", h=NHL)

                    # gates raw stash (sigmoid applied per chunk-group)
                    nc.vector.tensor_copy(out=gate_sb[:, i, :, :], in_=gview)

                    # fused rmsnorm for q heads + k
                    sq5 = p1small.tile([128, N5, HD], f32, tag="sq5")
                    nc.vector.tensor_tensor(sq5[:], qk, qk, op=ALU.mult)
                    ssq5 = p1small.tile([128, N5], f32, tag="ssq5")
                    nc.vector.tensor_reduce(
                        ssq5[:], sq5[:], axis=mybir.AxisListType.X, op=ALU.add
                    )
                    rstd5 = p1small.tile([128, N5], f32, tag="rstd5")
                    nc.scalar.activation(
                        out=rstd5[:], in_=ssq5[:], func=AF.Sqrt,
                        scale=1.0 / HD, bias=eps_sb[:],
                    )
                    nc.vector.reciprocal(rstd5[:], rstd5[:])
                    qkn = p1sb.tile([128, N5, HD], f32, tag="qkn")
                    nc.vector.tensor_tensor(
                        qkn[:], qk,
                        rstd5[:, :, None].to_broadcast([128, N5, HD]),
                        op=ALU.mult,
                    )
                    nc.vector.tensor_tensor(qkn[:], qkn[:], w1qk_sb[:], op=ALU.mult)

                    # rope into f32 rot + bf16 cast
                    cc = cos5_sb[:, i, :, :]
                    ss = sin5_sb[:, i, :, :]
                    x1 = qkn[:, :, 0:ROT:2]
                    x2 = qkn[:, :, 1:ROT:2]
                    rot = p1small.tile([128, N5, ROT], f32, tag="rot")
                    re = rot[:, :, 0:ROT:2]
                    ro = rot[:, :, 1:ROT:2]
                    t1 = p1small.tile([128, N5, RH], f32, tag="t1")
                    t2 = p1small.tile([128, N5, RH], f32, tag="t2")
                    nc.vector.tensor_tensor(t1[:], x2, ss, op=ALU.mult)
                    nc.vector.tensor_tensor(re, x1, cc, op=ALU.mult)
                    nc.vector.tensor_tensor(re, re, t1[:], op=ALU.subtract)
                    nc.vector.tensor_tensor(t2[:], x1, ss, op=ALU.mult)
                    nc.vector.tensor_tensor(ro, x2, cc, op=ALU.mult)
                    nc.vector.tensor_tensor(ro, ro, t2[:], op=ALU.add)

                    qk5b = p1sb.tile([128, N5, HD], bf16, tag="qk5b")
                    nc.vector.tensor_copy(out=qk5b[:, :, 0:ROT], in_=rot[:])
                    nc.vector.tensor_copy(
                        out=qk5b[:, :, ROT:HD], in_=qkn[:, :, ROT:HD]
                    )

                    nc.gpsimd.dma_start(
                        out=k_cache[i * 128 : (i + 1) * 128, 0:ROT],
                        in_=rot[:, NHL, :],
                    )
                    nc.gpsimd.dma_start(
                        out=k_cache[i * 128 : (i + 1) * 128, ROT:HD],
                        in_=qkn[:, NHL, ROT:HD],
                    )

                    for h in range(N5):
                        tp = pp_t.tile([128, 128], bf16, tag="tpb")
                        nc.tensor.transpose(tp[:], qk5b[:, h, :], ident_b[:])
                        dst = (
                            qT_sb[:, h, i * 128 : (i + 1) * 128]
                            if h < NHL
                            else kT_sb[:, i * 128 : (i + 1) * 128]
                        )
                        nc.vector.tensor_copy(out=dst, in_=tp[:])

                    nc.vector.tensor_copy(
                        out=v_sb[:, i, 0:HD], in_=raw[:, NQG + HD : NQG + 2 * HD]
                    )
                    nc.gpsimd.dma_start(
                        out=v_cache[i * 128 : (i + 1) * 128, :],
                        in_=raw[:, NQG + HD : NQG + 2 * HD],
                    )

                def do_scores(j, h):
                    ntb = TPB * (j + 1)
                    exp_tiles = []
                    for tb in range(ntb):
                        s_lo = max(j * BAND, tb * 128)
                        ne = (j + 1) * BAND - s_lo
                        ps = pp_s.tile([128, BAND], f32, tag="ps")
                        nc.tensor.matmul(
                            ps[:, :ne],
                            lhsT=kT_sb[:, tb * 128 : (tb + 1) * 128],
                            rhs=qT_sb[:, h, s_lo : (j + 1) * BAND],
                            start=True, stop=True,
                        )
                        et = exp_pool.tile([128, BAND], bf16, tag="expT")
                        nc.scalar.activation(
                            out=et[:, :ne], in_=ps[:, :ne],
                            func=AF.Exp, scale=SCALE,
                        )
                        if tb * 128 >= j * BAND:
                            nc.vector.tensor_tensor(
                                et[:, 0:128], et[:, 0:128], tri_sb[:],
                                op=ALU.mult,
                            )
                        exp_tiles.append((et, s_lo))
                    return exp_tiles

                def do_attnv(j, h, exp_tiles):
                    for sl in range(TPB):
                        sblk = j * TPB + sl
                        po = pp_o.tile([128, HD + 1], f32, tag="po")
                        for tb in range(sblk + 1):
                            et, s_lo = exp_tiles[tb]
                            co = sblk * 128 - s_lo
                            nc.tensor.matmul(
                                po[:],
                                lhsT=et[:, co : co + 128],
                                rhs=v_sb[:, tb, :],
                                start=(tb == 0),
                                stop=(tb == sblk),
                            )
                        rec = at_small.tile([128, 1], f32, tag="rec")
                        nc.vector.reciprocal(rec[:], po[:, HD : HD + 1])
                        ag = ag_pool.tile([128, HD], bf16, tag="ag")
                        nc.vector.tensor_scalar(
                            ag[:], po[:, 0:HD], rec[:], None, op0=ALU.mult
                        )
                        nc.vector.tensor_tensor(
                            ag[:], ag[:], gate_sb[:, sblk, h, :], op=ALU.mult
                        )
                        tp2 = pp_t.tile([128, 128], bf16, tag="tpb")
                        nc.tensor.transpose(tp2[:], ag[:], ident_b[:])
                        agb = ag_pool.tile([128, 128], bf16, tag="agb")
                        nc.vector.tensor_copy(out=agb[:], in_=tp2[:])
                        in_a = h < HSPLIT
                        a2a_in = a2a_in_a if in_a else a2a_in_b
                        hw_ = HSPLIT if in_a else NHL - HSPLIT
                        hh = h if in_a else h - HSPLIT
                        s0 = sblk * 128
                        jlo, jhi = s0 // SQ, (s0 + 127) // SQ
                        for jj in range(jlo, jhi + 1):
                            lo = max(s0, jj * SQ)
                            hi = min(s0 + 128, (jj + 1) * SQ)
                            dst = a2a_in[jj].rearrange(
                                "(p h) s -> p h s", h=hw_
                            )
                            nc.sync.dma_start(
                                out=dst[:, hh, lo - jj * SQ : hi - jj * SQ],
                                in_=agb[:, lo - s0 : hi - s0],
                            )

                # Sweep 1: chunk-groups interleaved with heads [0, HSPLIT)
                # so the first AllToAll can fire at ~60% of the kernel.
                for g in range(NBAND):
                    for i in range(TPB * g, TPB * (g + 1)):
                        do_chunk(i)
                    # this group's gate sigmoids, batched (one table stretch)
                    for i in range(TPB * g, TPB * (g + 1)):
                        nc.scalar.activation(
                            out=gate_sb[:, i, :, :], in_=gate_sb[:, i, :, :],
                            func=AF.Sigmoid,
                        )
                    prev = None
                    for h in range(HSPLIT):
                        tiles = do_scores(g, h)
                        if prev is not None:
                            do_attnv(g, prev[0], prev[1])
                        prev = (h, tiles)
                    do_attnv(g, prev[0], prev[1])
                nc.gpsimd.collective_compute(
                    "AllToAll", ALU.bypass,
                    replica_groups=[list(range(NC))],
                    ins=[a2a_in_a[:].opt()],
                    outs=[a2a_out_a[:].opt()],
                )
                # Sweep 2: remaining heads, pipelined across bands, while
                # the first AllToAll is in flight.
                prev = None
                for g in range(NBAND):
                    for h in range(HSPLIT, NHL):
                        tiles = do_scores(g, h)
                        if prev is not None:
                            do_attnv(prev[0], prev[1], prev[2])
                        prev = (g, h, tiles)
                do_attnv(prev[0], prev[1], prev[2])
                nc.gpsimd.collective_compute(
                    "AllToAll", ALU.bypass,
                    replica_groups=[list(range(NC))],
                    ins=[a2a_in_b[:].opt()],
                    outs=[a2a_out_b[:].opt()],
                )

            # ============ phase 4: o_proj ============
            # Split contraction in head-halves: pass A (a2a half a) runs while
            # the second AllToAll is still in flight; pass B adds on top.
            SL = (SQ + 127) // 128
            PSL = min(128, SQ)
            GB = NC // 2                      # kv-group blocks per batch

            with tc.tile_pool(name="agf_pool", bufs=1) as agf_pool, \
                 tc.tile_pool(name="wo_pool", bufs=NOB) as wo_pool, \
                 tc.tile_pool(name="op_out", bufs=NOB * 2 * SL + 1) as op_out, \
                 tc.tile_pool(name="pp_op", bufs=4, space="PSUM") as pp_op:

                wo_tiles = []
                for nb in range(NOB):
                    nw = min(512, HID - nb * 512)
                    wo_nb = wo_pool.tile([128, KO2, 512], bf16, tag="wo_nb")
                    wo_tiles.append(wo_nb)
                    nc.sync.dma_start(
                        out=wo_nb[:, :, :nw],
                        in_=wo[
                            :, nb * KO2 * 512 : (nb + 1) * KO2 * 512
                        ].rearrange("p (ko n) -> p ko n", ko=KO2),
                    )

                # gathered [p, batch, blk, h_half, sq] per half
                agf_a = agf_pool.tile([128, 2, GB, HSPLIT, SQ], bf16)
                agf_b = agf_pool.tile([128, 2, GB, NHL - HSPLIT, SQ], bf16)
                for bb in range(2):
                    for blk in range(GB):
                        nc.sync.dma_start(
                            out=agf_a[:, bb, blk, :, :],
                            in_=a2a_out_a[bb * GB + blk].rearrange(
                                "(p h) s -> p h s", h=HSPLIT
                            ),
                        )
                        nc.sync.dma_start(
                            out=agf_b[:, bb, blk, :, :],
                            in_=a2a_out_b[bb * GB + blk].rearrange(
                                "(p h) s -> p h s", h=NHL - HSPLIT
                            ),
                        )

                def lhs_for(bb, k2, sl):
                    blk, h = divmod(k2, NHL)
                    buf = agf_a if h < HSPLIT else agf_b
                    hh = h if h < HSPLIT else h - HSPLIT
                    return buf[:, bb, blk, hh, sl * 128 : sl * 128 + PSL]

                k2_a = [k2 for k2 in range(KO2) if k2 % NHL < HSPLIT]
                k2_b = [k2 for k2 in range(KO2) if k2 % NHL >= HSPLIT]


                of_tiles = {}
                for nb in range(NOB):          # pass A (needs only half a)
                    nw = min(512, HID - nb * 512)
                    for bb in range(2):
                        for sl in range(SL):
                            pso = pp_op.tile([PSL, 512], f32, tag="pso")
                            for idx, k2 in enumerate(k2_a):
                                nc.tensor.matmul(
                                    pso[:, :nw],
                                    lhsT=lhs_for(bb, k2, sl),
                                    rhs=wo_tiles[nb][:, k2, :nw],
                                    start=(idx == 0),
                                    stop=(idx == len(k2_a) - 1),
                                )
                            of = op_out.tile([PSL, 512], f32, tag="of")
                            of_tiles[(nb, bb, sl)] = of
                            nc.vector.tensor_copy(out=of[:, :nw], in_=pso[:, :nw])
                for nb in range(NOB):          # pass B (adds half b)
                    nw = min(512, HID - nb * 512)
                    for bb in range(2):
                        for sl in range(SL):
                            pso = pp_op.tile([PSL, 512], f32, tag="pso")
                            for idx, k2 in enumerate(k2_b):
                                nc.tensor.matmul(
                                    pso[:, :nw],
                                    lhsT=lhs_for(bb, k2, sl),
                                    rhs=wo_tiles[nb][:, k2, :nw],
                                    start=(idx == 0),
                                    stop=(idx == len(k2_b) - 1),
                                )
                            of = of_tiles[(nb, bb, sl)]
                            nc.vector.tensor_tensor(
                                of[:, :nw], of[:, :nw], pso[:, :nw], op=ALU.add
                            )
                            nc.gpsimd.dma_start(
                                out=out[
                                    bb, sl * 128 : sl * 128 + PSL,
                                    nb * 512 : nb * 512 + nw,
                                ],
                                in_=of[:, :nw],
                            )

    return nc


# ---------------------------------------------------------------- host side

def _pack_rows(a, p=128):
    """[R, N] with R = k*p  ->  [p, k*N] grouping rows by (k, p)."""
    R, N = a.shape
    k = R // p
    return np.ascontiguousarray(a.reshape(k, p, N).transpose(1, 0, 2).reshape(p, k * N))


def _shard_inputs(inputs, cfg):
    """Build per-core in_maps from the full problem inputs."""
    c = _derived(cfg)
    B, S, HID, NH, HD, ROT = c["B"], c["S"], c["HID"], c["NH"], c["HD"], c["ROT"]
    GC, NHL, NC, KO, SB = c["GC"], c["NHL"], c["NCORES"], c["KO"], c["SB"]
    KO2, NOB = c["KO2"], c["NOB"]
    N5 = NHL + 1
    RH = ROT // 2
    bf = ml_dtypes.bfloat16

    hs = np.asarray(inputs["hidden_states"], np.float32)
    wq = np.asarray(inputs["wq"], np.float32)
    wk = np.asarray(inputs["wk"], np.float32)
    wv = np.asarray(inputs["wv"], np.float32)
    wo = np.asarray(inputs["wo"], np.float32)
    q_norm_w = np.asarray(inputs["q_norm_w"], np.float32)
    k_norm_w = np.asarray(inputs["k_norm_w"], np.float32)
    rope_cos = np.ascontiguousarray(np.asarray(inputs["rope_cos"], np.float32)[:S])
    rope_sin = np.ascontiguousarray(np.asarray(inputs["rope_sin"], np.float32)[:S])

    # wo packed per n-band: [p, nb, ko2, n']
    wo_b = (
        wo.astype(bf)
        .reshape(KO2, 128, NOB, 512)
        .transpose(1, 2, 0, 3)
        .reshape(128, NOB * KO2 * 512)
    )
    wo_b = np.ascontiguousarray(wo_b)

    w1qk = np.concatenate(
        [np.tile(1.0 + q_norm_w, NHL), 1.0 + k_norm_w]
    )[None, :].repeat(128, 0).copy()

    # rope tables packed [p, sb, h(=N5 copies), f]
    def pack_rope(t):
        r = t.reshape(SB, 128, RH).transpose(1, 0, 2)          # [p, sb, f]
        r = np.repeat(r[:, :, None, :], N5, axis=2)            # [p, sb, h, f]
        return np.ascontiguousarray(r.reshape(128, SB * N5 * RH))

    cos5 = pack_rope(rope_cos)
    sin5 = pack_rope(rope_sin)

    tt = np.arange(128)
    tri = (tt[None, :] >= tt[:, None]).astype(bf)

    in_maps = []
    for core in range(NC):
        b, g = divmod(core, GC)
        # x^T packed per s-chunk: [p, sb, ko, s_lo]
        xb = hs[b].T.astype(bf)                                # [HID, S]
        xTc = np.ascontiguousarray(
            xb.reshape(KO, 128, SB, 128)
            .transpose(1, 2, 0, 3)
            .reshape(128, SB * KO * 128)
        )
        # wq columns reordered to [q0..q3 | gate0..gate3], then row-packed
        wq_g = wq[:, g * NHL * 2 * HD : (g + 1) * NHL * 2 * HD]
        wq_r = wq_g.reshape(HID, NHL, 2, HD)
        wq_dev = np.concatenate(
            [wq_r[:, :, 0, :].reshape(HID, NHL * HD),
             wq_r[:, :, 1, :].reshape(HID, NHL * HD)], axis=1
        ).astype(bf)
        wkv_dev = np.concatenate(
            [wk[:, g * HD : (g + 1) * HD], wv[:, g * HD : (g + 1) * HD]],
            axis=1,
        ).astype(bf)
        in_maps.append(
            dict(
                xT=xTc,
                wq=_pack_rows(wq_dev),
                wkv=_pack_rows(wkv_dev),
                wo=wo_b,
                cos5=cos5, sin5=sin5, w1qk=w1qk, tri=tri,
            )
        )
    return in_maps


def assemble_outputs(results, cfg):
    c = _derived(cfg)
    B, S, HID, HD = c["B"], c["S"], c["HID"], c["HD"]
    GC, NC, SQ, NKV = c["GC"], c["NCORES"], c["SQ"], c["NKV"]

    out = np.empty((B, S, HID), np.float32)
    k_cache = np.empty((B, NKV, S, HD), np.float32)
    v_cache = np.empty((B, NKV, S, HD), np.float32)
    for core in range(NC):
        r = results[core]
        b, g = divmod(core, GC)
        out[:, core * SQ : (core + 1) * SQ, :] = np.asarray(r["out"]).reshape(
            B, SQ, HID
        )
        k_cache[b, g] = np.asarray(r["k_cache"]).reshape(S, HD)
        v_cache[b, g] = np.asarray(r["v_cache"]).reshape(S, HD)
    return out, k_cache, v_cache


_NC_CACHE = {}


def kernel(**inputs):
    from concourse.bass_utils import run_bass_kernel_spmd

    cfg = FULL_CFG
    key = "full"
    if key not in _NC_CACHE:
        nc = build_nc(cfg)
        nc.finalize()
        _NC_CACHE[key] = nc
    nc = _NC_CACHE[key]

    in_maps = _shard_inputs(inputs, cfg)
    trace = bool(int(os.environ.get("KERNEL_TRACE", "0")))
    res = run_bass_kernel_spmd(
        nc, in_maps, core_ids=list(range(_derived(cfg)["NCORES"])), trace=trace
    )
    if trace and res.exec_time_ns is not None:
        print(f"HW exec time: {res.exec_time_ns} ns", flush=True)
        kernel.last_exec_time_ns = res.exec_time_ns
    return assemble_outputs(res.results, cfg)


# revision 30
# speedup vs baseline: 1.1867x; 1.0526x over previous
"""Distributed Bass kernel for fused GQA attention block (ANEFullAttention).

Full op: qkv-proj (wq also produces a sigmoid gate), q/k rmsnorm, partial
interleaved RoPE (first 32 of 128 dims), causal GQA attention (16 q heads /
4 kv heads), gate multiply, o_proj; returns (out, k_cache, v_cache).

Sharding over 8 cores: core c -> (batch b = c//4, head-group g = c%4).
Each core owns 4 q heads + 1 kv head of one batch.  o_proj is handled by an
8-way AllToAll (split in two halves by head pair, so the first half overlaps
the remaining attention work): each core ships its gated attention output
(transposed, [d_local, S]) sliced into 8 s-shards; afterwards core j holds
the full 2048-dim attention output for BOTH batches on s-rows
[j*S/8,(j+1)*S/8) and computes that slice of o_proj against replicated wo.

Perf notes (measured on TRN2):
- All weight/activation DRAM parameters are pre-packed on the host into the
  exact [128, ...contiguous-free-dim] device layout so every HWDGE dma_start
  is a cheap 2D pattern (multi-dim APs are descriptor-generated inline on
  the issuing sequencer at ~5-12us per call).
- ScalarE(ACT) runs only Exp + batched Sigmoid/Sqrt (activation-table swaps
  cost ~1.3us); DVE does PSUM evictions, norm scaling, rope, masks, gating.
- q heads and the kv head share one fused norm/rope pipeline (wq columns are
  host-reordered to [q0..q3 | gate0..gate3] so q|k rows are contiguous).
- walrus runs with ldw-opt disabled: every matmul pays its own LDWEIGHTS,
  so matmul count is kept minimal and moving dims maximal.
"""

import os
import sys

_TRN_REPO = "/opt/trn_rl_repo"
if _TRN_REPO not in sys.path:
    sys.path.insert(0, _TRN_REPO)

import numpy as np
import ml_dtypes

# ---------------------------------------------------------------- config

FULL_CFG = dict(
    B=2, S=2048, HID=2048, NH=16, NKV=4, HD=128, ROT=32,
    THETA=10000000.0, EPS=1e-6,
)


def _derived(cfg):
    d = dict(cfg)
    d["GC"] = 4                       # head-groups (tensor-parallel degree)
    d["NCORES"] = 8
    d["NHL"] = d["NH"] // d["GC"]     # q heads per core
    d["KO"] = d["HID"] // 128         # contraction blocks for projections
    d["SB"] = d["S"] // 128           # 128-row s blocks
    d["BAND"] = min(512, d["S"])      # moving-dim width for score matmuls
    d["NBAND"] = d["S"] // d["BAND"]
    d["TPB"] = d["BAND"] // 128       # t-blocks per band
    d["SQ"] = d["S"] // d["NCORES"]   # per-core o_proj s-slice (per batch)
    d["DM"] = d["NH"] * d["HD"]       # attention model dim (o_proj contraction)
    d["KO2"] = d["DM"] // 128
    d["NOB"] = (d["HID"] + 511) // 512
    d["SCALE"] = d["HD"] ** -0.5
    return d


# ---------------------------------------------------------------- builder

def build_nc(cfg):
    import concourse.bass as bass
    import concourse.tile as tile
    import concourse.mybir as mybir
    from concourse import bacc
    from concourse.masks import make_identity

    c = _derived(cfg)
    S, HID, HD, ROT = c["S"], c["HID"], c["HD"], c["ROT"]
    NHL, KO, SB = c["NHL"], c["KO"], c["SB"]
    BAND, NBAND, TPB, SQ = c["BAND"], c["NBAND"], c["TPB"], c["SQ"]
    KO2, NOB, SCALE, EPS = c["KO2"], c["NOB"], c["SCALE"], c["EPS"]
    NC = c["NCORES"]
    NQG = NHL * 2 * HD                # 1024 (q heads then gates)
    N5 = NHL + 1                      # q heads + the kv head
    RH = ROT // 2

    f32 = mybir.dt.float32
    bf16 = mybir.dt.bfloat16
    AF = mybir.ActivationFunctionType
    ALU = mybir.AluOpType

    nc = bacc.Bacc(None, target_bir_lowering=False, debug=False, num_devices=NC)

    # -------- dram parameters (names = in_map keys; all pre-packed 2D)
    xT = nc.declare_dram_parameter("xT", [128, SB * KO * 128], bf16, isOutput=False)
    wq = nc.declare_dram_parameter("wq", [128, KO * NQG], bf16, isOutput=False)
    wkv = nc.declare_dram_parameter("wkv", [128, KO * 2 * HD], bf16, isOutput=False)
    wo = nc.declare_dram_parameter("wo", [128, NOB * KO2 * 512], bf16, isOutput=False)
    cos5 = nc.declare_dram_parameter("cos5", [128, SB * N5 * RH], f32, isOutput=False)
    sin5 = nc.declare_dram_parameter("sin5", [128, SB * N5 * RH], f32, isOutput=False)
    w1qk = nc.declare_dram_parameter("w1qk", [128, N5 * HD], f32, isOutput=False)
    tri = nc.declare_dram_parameter("tri", [128, 128], bf16, isOutput=False)

    out = nc.declare_dram_parameter("out", [2, SQ, HID], f32, isOutput=True)
    k_cache = nc.declare_dram_parameter("k_cache", [S, HD], f32, isOutput=True)
    v_cache = nc.declare_dram_parameter("v_cache", [S, HD], f32, isOutput=True)

    with tile.TileContext(nc) as tc:
        with tc.tile_pool(name="const", bufs=1) as const, \
             tc.tile_pool(name="persist", bufs=1) as persist, \
             tc.tile_pool(name="dram", bufs=1, space="DRAM") as dram:
            ident_b = const.tile([128, 128], bf16)
            make_identity(nc, ident_b)

            w1qk_sb = const.tile([128, N5, HD], f32)
            nc.gpsimd.dma_start(
                out=w1qk_sb[:], in_=w1qk.rearrange("p (h d) -> p h d", h=N5)
            )
            tri_sb = const.tile([128, 128], bf16)
            nc.gpsimd.dma_start(out=tri_sb[:], in_=tri[:, :])
            eps_sb = const.tile([128, 1], f32)
            nc.vector.memset(eps_sb[:], EPS)

            cos5_sb = const.tile([128, SB, N5, RH], f32)
            nc.gpsimd.dma_start(
                out=cos5_sb[:],
                in_=cos5.rearrange("p (sb h f) -> p sb h f", sb=SB, h=N5),
            )
            sin5_sb = const.tile([128, SB, N5, RH], f32)
            nc.gpsimd.dma_start(
                out=sin5_sb[:],
                in_=sin5.rearrange("p (sb h f) -> p sb h f", sb=SB, h=N5),
            )

            # phase-1 -> phase-2 tensors
            qT_sb = persist.tile([128, NHL, S], bf16)      # [d, h, s]
            kT_sb = persist.tile([128, S], bf16)           # [d, t]
            v_sb = persist.tile([128, SB, HD + 1], bf16)   # [t_lo, t_blk, d+ones]
            gate_sb = persist.tile([128, SB, NHL, HD], bf16)
            nc.vector.memset(v_sb[:, :, HD : HD + 1], 1.0)

            # a2a bounce buffers, split in half by head; rows (p*HSPLIT + h)
            # so the o_proj gather is a 2D-contiguous DMA per block.
            HSPLIT = max(1, NHL // 2)
            HH = HSPLIT * HD
            a2a_in_a = dram.tile([NC, HH, SQ], bf16)
            a2a_out_a = dram.tile([NC, HH, SQ], bf16)
            a2a_in_b = dram.tile([NC, HH, SQ], bf16)
            a2a_out_b = dram.tile([NC, HH, SQ], bf16)

            # ============ phases 1+2 interleaved =========================
            # Attention band j only needs the first TPB*(j+1) s-chunks, so
            # projection chunk-groups and attention bands share one TensorE
            # stream: while TensorE runs a chunk group's matmuls, ScalarE
            # drains the previous band's Exp backlog (attention alone is
            # ACT-bound).  Within a band, heads are software-pipelined depth
            # 1: scores(h+1) are issued before attn@v(h) so attn@v never
            # waits on a cold Exp.  All PSUM pools coexist: acc 3 (qg0, qg1,
            # kv share one tag) + scores 2 + attn-out 2 + transpose 1 = 8.
            with tc.tile_pool(name="wq_pool", bufs=1) as wq_pool, \
                 tc.tile_pool(name="xs_pool", bufs=3) as xs_pool, \
                 tc.tile_pool(name="p1sb", bufs=3) as p1sb, \
                 tc.tile_pool(name="p1small", bufs=6) as p1small, \
                 tc.tile_pool(name="exp_pool", bufs=2 * SB + 2) as exp_pool, \
                 tc.tile_pool(name="ag_pool", bufs=4) as ag_pool, \
                 tc.tile_pool(name="at_small", bufs=8) as at_small, \
                 tc.tile_pool(name="pp_a", bufs=3, space="PSUM") as pp_a, \
                 tc.tile_pool(name="pp_s", bufs=2, space="PSUM") as pp_s, \
                 tc.tile_pool(name="pp_o", bufs=2, space="PSUM") as pp_o, \
                 tc.tile_pool(name="pp_t", bufs=1, space="PSUM") as pp_t:

                wq_sb = wq_pool.tile([128, KO, NQG], bf16)
                wkv_sb = wq_pool.tile([128, KO, 2 * HD], bf16)
                # first chunks split fine so several DMA queues fill in
                # parallel and the first matmul starts early
                splits = [1, 1, 2, 4] + [4] * KO
                kq = 0
                while kq < KO:
                    KQ = min(splits.pop(0), KO - kq)
                    nc.scalar.dma_start(
                        out=wq_sb[:, kq : kq + KQ, :],
                        in_=wq[:, kq * NQG : (kq + KQ) * NQG],
                    )
                    kq += KQ
                nc.scalar.dma_start(
                    out=wkv_sb[:],
                    in_=wkv.rearrange("p (ko n) -> p ko n", ko=KO),
                )

                n_qg = (NQG + 511) // 512  # psum tiles per s-chunk (512 each)
                CW = KO * 128              # xT columns per s-chunk

                def do_chunk(i):
                    xs = xs_pool.tile([128, KO, 128], bf16, tag="xs")
                    kh = KO // 2
                    nc.sync.dma_start(
                        out=xs[:, 0:kh, :],
                        in_=xT[:, i * CW : i * CW + CW // 2],
                    )
                    nc.sync.dma_start(
                        out=xs[:, kh:KO, :],
                        in_=xT[:, i * CW + CW // 2 : (i + 1) * CW],
                    )

                    qg_ps = []
                    for n2 in range(n_qg):
                        w = min(512, NQG - n2 * 512)
                        ps = pp_a.tile([128, w], f32, tag="acc",
                                       padded_shape=[128, 512])
                        qg_ps.append(ps)
                        for ko in range(KO):
                            nc.tensor.matmul(
                                ps[:],
                                lhsT=xs[:, ko, :],
                                rhs=wq_sb[:, ko, n2 * 512 : n2 * 512 + w],
                                start=(ko == 0),
                                stop=(ko == KO - 1),
                            )
                    kv_ps = pp_a.tile([128, 2 * HD], f32, tag="acc",
                                      padded_shape=[128, 512])
                    for ko in range(KO):
                        nc.tensor.matmul(
                            kv_ps[:],
                            lhsT=xs[:, ko, :],
                            rhs=wkv_sb[:, ko, :],
                            start=(ko == 0),
                            stop=(ko == KO - 1),
                        )

                    # evict to one combined tile: [q(NHL*HD) | k | gate | v]
                    QW = NHL * HD
                    raw = p1sb.tile([128, NQG + 2 * HD], f32, tag="raw")
                    for n2, ps in enumerate(qg_ps):
                        lo = n2 * 512
                        hi = lo + ps.shape[1]
                        if lo < QW:
                            e = min(hi, QW)
                            nc.vector.tensor_copy(
                                out=raw[:, lo:e], in_=ps[:, 0 : e - lo]
                            )
                        if hi > QW:
                            s0 = max(lo, QW)
                            nc.vector.tensor_copy(
                                out=raw[:, s0 + HD : hi + HD],
                                in_=ps[:, s0 - lo : hi - lo],
                            )
                    nc.vector.tensor_copy(
                        out=raw[:, QW : QW + HD], in_=kv_ps[:, 0:HD]
                    )
                    nc.vector.tensor_copy(
                        out=raw[:, NQG + HD : NQG + 2 * HD],
                        in_=kv_ps[:, HD : 2 * HD],
                    )
                    qk = raw[:, 0 : N5 * HD].rearrange(
                        "p (h d) -> p h d", h=N5
                    )
                    gview = raw[
                        :, QW + HD : QW + HD + QW
                    ].rearrange("p (h d) -> p h d", h=NHL)

                    # gates raw stash (sigmoid applied per chunk-group)
                    nc.vector.tensor_copy(out=gate_sb[:, i, :, :], in_=gview)

                    # fused rmsnorm for q heads + k
                    sq5 = p1small.tile([128, N5, HD], f32, tag="sq5")
                    nc.vector.tensor_tensor(sq5[:], qk, qk, op=ALU.mult)
                    ssq5 = p1small.tile([128, N5], f32, tag="ssq5")
                    nc.vector.tensor_reduce(
                        ssq5[:], sq5[:], axis=mybir.AxisListType.X, op=ALU.add
                    )
                    rstd5 = p1small.tile([128, N5], f32, tag="rstd5")
                    nc.scalar.activation(
                        out=rstd5[:], in_=ssq5[:], func=AF.Sqrt,
                        scale=1.0 / HD, bias=eps_sb[:],
                    )
                    nc.vector.reciprocal(rstd5[:], rstd5[:])
                    qkn = p1sb.tile([128, N5, HD], f32, tag="qkn")
                    nc.vector.tensor_tensor(
                        qkn[:], qk,
                        rstd5[:, :, None].to_broadcast([128, N5, HD]),
                        op=ALU.mult,
                    )
                    nc.vector.tensor_tensor(qkn[:], qkn[:], w1qk_sb[:], op=ALU.mult)

                    # rope into f32 rot + bf16 cast
                    cc = cos5_sb[:, i, :, :]
                    ss = sin5_sb[:, i, :, :]
                    x1 = qkn[:, :, 0:ROT:2]
                    x2 = qkn[:, :, 1:ROT:2]
                    rot = p1small.tile([128, N5, ROT], f32, tag="rot")
                    re = rot[:, :, 0:ROT:2]
                    ro = rot[:, :, 1:ROT:2]
                    t1 = p1small.tile([128, N5, RH], f32, tag="t1")
                    t2 = p1small.tile([128, N5, RH], f32, tag="t2")
                    nc.vector.tensor_tensor(t1[:], x2, ss, op=ALU.mult)
                    nc.vector.tensor_tensor(re, x1, cc, op=ALU.mult)
                    nc.vector.tensor_tensor(re, re, t1[:], op=ALU.subtract)
                    nc.vector.tensor_tensor(t2[:], x1, ss, op=ALU.mult)
                    nc.vector.tensor_tensor(ro, x2, cc, op=ALU.mult)
                    nc.vector.tensor_tensor(ro, ro, t2[:], op=ALU.add)

                    qk5b = p1sb.tile([128, N5, HD], bf16, tag="qk5b")
                    nc.vector.tensor_copy(out=qk5b[:, :, 0:ROT], in_=rot[:])
                    nc.vector.tensor_copy(
                        out=qk5b[:, :, ROT:HD], in_=qkn[:, :, ROT:HD]
                    )

                    nc.gpsimd.dma_start(
                        out=k_cache[i * 128 : (i + 1) * 128, 0:ROT],
                        in_=rot[:, NHL, :],
                    )
                    nc.gpsimd.dma_start(
                        out=k_cache[i * 128 : (i + 1) * 128, ROT:HD],
                        in_=qkn[:, NHL, ROT:HD],
                    )

                    for h in range(N5):
                        tp = pp_t.tile([128, 128], bf16, tag="tpb")
                        nc.tensor.transpose(tp[:], qk5b[:, h, :], ident_b[:])
                        dst = (
                            qT_sb[:, h, i * 128 : (i + 1) * 128]
                            if h < NHL
                            else kT_sb[:, i * 128 : (i + 1) * 128]
                        )
                        nc.vector.tensor_copy(out=dst, in_=tp[:])

                    nc.vector.tensor_copy(
                        out=v_sb[:, i, 0:HD], in_=raw[:, NQG + HD : NQG + 2 * HD]
                    )
                    nc.gpsimd.dma_start(
                        out=v_cache[i * 128 : (i + 1) * 128, :],
                        in_=raw[:, NQG + HD : NQG + 2 * HD],
                    )

                def do_scores(j, h):
                    ntb = TPB * (j + 1)
                    exp_tiles = []
                    for tb in range(ntb):
                        s_lo = max(j * BAND, tb * 128)
                        ne = (j + 1) * BAND - s_lo
                        ps = pp_s.tile([128, BAND], f32, tag="ps")
                        nc.tensor.matmul(
                            ps[:, :ne],
                            lhsT=kT_sb[:, tb * 128 : (tb + 1) * 128],
                            rhs=qT_sb[:, h, s_lo : (j + 1) * BAND],
                            start=True, stop=True,
                        )
                        et = exp_pool.tile([128, BAND], bf16, tag="expT")
                        nc.scalar.activation(
                            out=et[:, :ne], in_=ps[:, :ne],
                            func=AF.Exp, scale=SCALE,
                        )
                        if tb * 128 >= j * BAND:
                            nc.vector.tensor_tensor(
                                et[:, 0:128], et[:, 0:128], tri_sb[:],
                                op=ALU.mult,
                            )
                        exp_tiles.append((et, s_lo))
                    return exp_tiles

                def do_attnv(j, h, exp_tiles):
                    for sl in range(TPB):
                        sblk = j * TPB + sl
                        po = pp_o.tile([128, HD + 1], f32, tag="po")
                        for tb in range(sblk + 1):
                            et, s_lo = exp_tiles[tb]
                            co = sblk * 128 - s_lo
                            nc.tensor.matmul(
                                po[:],
                                lhsT=et[:, co : co + 128],
                                rhs=v_sb[:, tb, :],
                                start=(tb == 0),
                                stop=(tb == sblk),
                            )
                        rec = at_small.tile([128, 1], f32, tag="rec")
                        nc.vector.reciprocal(rec[:], po[:, HD : HD + 1])
                        ag = ag_pool.tile([128, HD], bf16, tag="ag")
                        nc.vector.tensor_scalar(
                            ag[:], po[:, 0:HD], rec[:], None, op0=ALU.mult
                        )
                        nc.vector.tensor_tensor(
                            ag[:], ag[:], gate_sb[:, sblk, h, :], op=ALU.mult
                        )
                        tp2 = pp_t.tile([128, 128], bf16, tag="tpb")
                        nc.tensor.transpose(tp2[:], ag[:], ident_b[:])
                        agb = ag_pool.tile([128, 128], bf16, tag="agb")
                        nc.vector.tensor_copy(out=agb[:], in_=tp2[:])
                        in_a = h < HSPLIT
                        a2a_in = a2a_in_a if in_a else a2a_in_b
                        hw_ = HSPLIT if in_a else NHL - HSPLIT
                        hh = h if in_a else h - HSPLIT
                        s0 = sblk * 128
                        jlo, jhi = s0 // SQ, (s0 + 127) // SQ
                        for jj in range(jlo, jhi + 1):
                            lo = max(s0, jj * SQ)
                            hi = min(s0 + 128, (jj + 1) * SQ)
                            dst = a2a_in[jj].rearrange(
                                "(p h) s -> p h s", h=hw_
                            )
                            nc.sync.dma_start(
                                out=dst[:, hh, lo - jj * SQ : hi - jj * SQ],
                                in_=agb[:, lo - s0 : hi - s0],
                            )

                # Sweep 1: chunk-groups interleaved with heads [0, HSPLIT)
                # so the first AllToAll can fire at ~60% of the kernel.
                for g in range(NBAND):
                    for i in range(TPB * g, TPB * (g + 1)):
                        do_chunk(i)
                    # this group's gate sigmoids, batched (one table stretch)
                    for i in range(TPB * g, TPB * (g + 1)):
                        nc.scalar.activation(
                            out=gate_sb[:, i, :, :], in_=gate_sb[:, i, :, :],
                            func=AF.Sigmoid,
                        )
                    prev = None
                    for h in range(HSPLIT):
                        tiles = do_scores(g, h)
                        if prev is not None:
                            do_attnv(g, prev[0], prev[1])
                        prev = (h, tiles)
                    do_attnv(g, prev[0], prev[1])
                nc.gpsimd.collective_compute(
                    "AllToAll", ALU.bypass,
                    replica_groups=[list(range(NC))],
                    ins=[a2a_in_a[:].opt()],
                    outs=[a2a_out_a[:].opt()],
                )
                # Sweep 2: remaining heads, pipelined across bands, while
                # the first AllToAll is in flight.
                prev = None
                for g in range(NBAND):
                    for h in range(HSPLIT, NHL):
                        tiles = do_scores(g, h)
                        if prev is not None:
                            do_attnv(prev[0], prev[1], prev[2])
                        prev = (g, h, tiles)
                do_attnv(prev[0], prev[1], prev[2])
                nc.gpsimd.collective_compute(
                    "AllToAll", ALU.bypass,
                    replica_groups=[list(range(NC))],
                    ins=[a2a_in_b[:].opt()],
                    outs=[a2a_out_b[:].opt()],
                )

            # ============ phase 4: o_proj ============
            # Split contraction in head-halves: pass A (a2a half a) runs while
            # the second AllToAll is still in flight; pass B adds on top.
            SL = (SQ + 127) // 128
            PSL = min(128, SQ)
            GB = NC // 2                      # kv-group blocks per batch

            with tc.tile_pool(name="agf_pool", bufs=1) as agf_pool, \
                 tc.tile_pool(name="wo_pool", bufs=NOB) as wo_pool, \
                 tc.tile_pool(name="op_out", bufs=NOB * 2 * SL + 1) as op_out, \
                 tc.tile_pool(name="pp_op", bufs=4, space="PSUM") as pp_op:

                wo_tiles = []
                for nb in range(NOB):
                    nw = min(512, HID - nb * 512)
                    wo_nb = wo_pool.tile([128, KO2, 512], bf16, tag="wo_nb")
                    wo_tiles.append(wo_nb)
                    nc.sync.dma_start(
                        out=wo_nb[:, :, :nw],
                        in_=wo[
                            :, nb * KO2 * 512 : (nb + 1) * KO2 * 512
                        ].rearrange("p (ko n) -> p ko n", ko=KO2),
                    )

                # gathered [p, batch, blk, h_half, sq] per half
                agf_a = agf_pool.tile([128, 2, GB, HSPLIT, SQ], bf16)
                agf_b = agf_pool.tile([128, 2, GB, NHL - HSPLIT, SQ], bf16)
                for bb in range(2):
                    for blk in range(GB):
                        nc.sync.dma_start(
                            out=agf_a[:, bb, blk, :, :],
                            in_=a2a_out_a[bb * GB + blk].rearrange(
                                "(p h) s -> p h s", h=HSPLIT
                            ),
                        )
                        nc.sync.dma_start(
                            out=agf_b[:, bb, blk, :, :],
                            in_=a2a_out_b[bb * GB + blk].rearrange(
                                "(p h) s -> p h s", h=NHL - HSPLIT
                            ),
                        )

                def lhs_for(bb, k2, sl):
                    blk, h = divmod(k2, NHL)
                    buf = agf_a if h < HSPLIT else agf_b
                    hh = h if h < HSPLIT else h - HSPLIT
                    return buf[:, bb, blk, hh, sl * 128 : sl * 128 + PSL]

                k2_a = [k2 for k2 in range(KO2) if k2 % NHL < HSPLIT]
                k2_b = [k2 for k2 in range(KO2) if k2 % NHL >= HSPLIT]


                of_tiles = {}
                for nb in range(NOB):          # pass A (needs only half a)
                    nw = min(512, HID - nb * 512)
                    for bb in range(2):
                        for sl in range(SL):
                            pso = pp_op.tile([PSL, 512], f32, tag="pso")
                            for idx, k2 in enumerate(k2_a):
                                nc.tensor.matmul(
                                    pso[:, :nw],
                                    lhsT=lhs_for(bb, k2, sl),
                                    rhs=wo_tiles[nb][:, k2, :nw],
                                    start=(idx == 0),
                                    stop=(idx == len(k2_a) - 1),
                                )
                            of = op_out.tile([PSL, 512], f32, tag="of")
                            of_tiles[(nb, bb, sl)] = of
                            nc.vector.tensor_copy(out=of[:, :nw], in_=pso[:, :nw])
                for nb in range(NOB):          # pass B (adds half b)
                    nw = min(512, HID - nb * 512)
                    for bb in range(2):
                        for sl in range(SL):
                            pso = pp_op.tile([PSL, 512], f32, tag="pso")
                            for idx, k2 in enumerate(k2_b):
                                nc.tensor.matmul(
                                    pso[:, :nw],
                                    lhsT=lhs_for(bb, k2, sl),
                                    rhs=wo_tiles[nb][:, k2, :nw],
                                    start=(idx == 0),
                                    stop=(idx == len(k2_b) - 1),
                                )
                            of = of_tiles[(nb, bb, sl)]
                            nc.vector.tensor_tensor(
                                of[:, :nw], of[:, :nw], pso[:, :nw], op=ALU.add
                            )
                            nc.gpsimd.dma_start(
                                out=out[
                                    bb, sl * 128 : sl * 128 + PSL,
                                    nb * 512 : nb * 512 + nw,
                                ],
                                in_=of[:, :nw],
                            )

    return nc


# ---------------------------------------------------------------- host side

def _pack_rows(a, p=128):
    """[R, N] with R = k*p  ->  [p, k*N] grouping rows by (k, p)."""
    R, N = a.shape
    k = R // p
    return np.ascontiguousarray(a.reshape(k, p, N).transpose(1, 0, 2).reshape(p, k * N))


def _shard_inputs(inputs, cfg):
    """Build per-core in_maps from the full problem inputs."""
    c = _derived(cfg)
    B, S, HID, NH, HD, ROT = c["B"], c["S"], c["HID"], c["NH"], c["HD"], c["ROT"]
    GC, NHL, NC, KO, SB = c["GC"], c["NHL"], c["NCORES"], c["KO"], c["SB"]
    KO2, NOB = c["KO2"], c["NOB"]
    N5 = NHL + 1
    RH = ROT // 2
    bf = ml_dtypes.bfloat16

    hs = np.asarray(inputs["hidden_states"], np.float32)
    wq = np.asarray(inputs["wq"], np.float32)
    wk = np.asarray(inputs["wk"], np.float32)
    wv = np.asarray(inputs["wv"], np.float32)
    wo = np.asarray(inputs["wo"], np.float32)
    q_norm_w = np.asarray(inputs["q_norm_w"], np.float32)
    k_norm_w = np.asarray(inputs["k_norm_w"], np.float32)
    rope_cos = np.ascontiguousarray(np.asarray(inputs["rope_cos"], np.float32)[:S])
    rope_sin = np.ascontiguousarray(np.asarray(inputs["rope_sin"], np.float32)[:S])

    # wo packed per n-band: [p, nb, ko2, n']
    wo_b = (
        wo.astype(bf)
        .reshape(KO2, 128, NOB, 512)
        .transpose(1, 2, 0, 3)
        .reshape(128, NOB * KO2 * 512)
    )
    wo_b = np.ascontiguousarray(wo_b)

    w1qk = np.concatenate(
        [np.tile(1.0 + q_norm_w, NHL), 1.0 + k_norm_w]
    )[None, :].repeat(128, 0).copy()

    # rope tables packed [p, sb, h(=N5 copies), f]
    def pack_rope(t):
        r = t.reshape(SB, 128, RH).transpose(1, 0, 2)          # [p, sb, f]
        r = np.repeat(r[:, :, None, :], N5, axis=2)            # [p, sb, h, f]
        return np.ascontiguousarray(r.reshape(128, SB * N5 * RH))

    cos5 = pack_rope(rope_cos)
    sin5 = pack_rope(rope_sin)

    tt = np.arange(128)
    tri = (tt[None, :] >= tt[:, None]).astype(bf)

    in_maps = []
    for core in range(NC):
        b, g = divmod(core, GC)
        # x^T packed per s-chunk: [p, sb, ko, s_lo]
        xb = hs[b].T.astype(bf)                                # [HID, S]
        xTc = np.ascontiguousarray(
            xb.reshape(KO, 128, SB, 128)
            .transpose(1, 2, 0, 3)
            .reshape(128, SB * KO * 128)
        )
        # wq columns reordered to [q0..q3 | gate0..gate3], then row-packed
        wq_g = wq[:, g * NHL * 2 * HD : (g + 1) * NHL * 2 * HD]
        wq_r = wq_g.reshape(HID, NHL, 2, HD)
        wq_dev = np.concatenate(
            [wq_r[:, :, 0, :].reshape(HID, NHL * HD),
             wq_r[:, :, 1, :].reshape(HID, NHL * HD)], axis=1
        ).astype(bf)
        wkv_dev = np.concatenate(
            [wk[:, g * HD : (g + 1) * HD], wv[:, g * HD : (g + 1) * HD]],
            axis=1,
        ).astype(bf)
        in_maps.append(
            dict(
                xT=xTc,
                wq=_pack_rows(wq_dev),
                wkv=_pack_rows(wkv_dev),
                wo=wo_b,
                cos5=cos5, sin5=sin5, w1qk=w1qk, tri=tri,
            )
        )
    return in_maps


def assemble_outputs(results, cfg):
    c = _derived(cfg)
    B, S, HID, HD = c["B"], c["S"], c["HID"], c["HD"]
    GC, NC, SQ, NKV = c["GC"], c["NCORES"], c["SQ"], c["NKV"]

    out = np.empty((B, S, HID), np.float32)
    k_cache = np.empty((B, NKV, S, HD), np.float32)
    v_cache = np.empty((B, NKV, S, HD), np.float32)
    for core in range(NC):
        r = results[core]
        b, g = divmod(core, GC)
        out[:, core * SQ : (core + 1) * SQ, :] = np.asarray(r["out"]).reshape(
            B, SQ, HID
        )
        k_cache[b, g] = np.asarray(r["k_cache"]).reshape(S, HD)
        v_cache[b, g] = np.asarray(r["v_cache"]).reshape(S, HD)
    return out, k_cache, v_cache


_NC_CACHE = {}


def kernel(**inputs):
    from concourse.bass_utils import run_bass_kernel_spmd

    cfg = FULL_CFG
    key = "full"
    if key not in _NC_CACHE:
        nc = build_nc(cfg)
        nc.finalize()
        _NC_CACHE[key] = nc
    nc = _NC_CACHE[key]

    in_maps = _shard_inputs(inputs, cfg)
    trace = bool(int(os.environ.get("KERNEL_TRACE", "0")))
    res = run_bass_kernel_spmd(
        nc, in_maps, core_ids=list(range(_derived(cfg)["NCORES"])), trace=trace
    )
    if trace and res.exec_time_ns is not None:
        print(f"HW exec time: {res.exec_time_ns} ns", flush=True)
        kernel.last_exec_time_ns = res.exec_time_ns
    return assemble_outputs(res.results, cfg)
